# revision 1
# baseline (speedup 1.0000x reference)
"""ChemConv (GNN message passing) kernel for Trainium2, 8 NeuronCores.

Reference math (per sample s):
    node_conn[a,f,d] = sum_n conn[a,n,f] * node[n,d]
    out[a,o]         = sum_{f,d} cat(node_conn, bond)[a,f,d] * filters[o,f,d]

Folded form used on device (filters folded into node features):
    W[n,f,o]  = sum_d node[n,d] * filters[o,f,d]          (tiny matmuls)
    out[a,o]  = sum_{n,f} conn[a,(n,f)] * W[(n,f),o]
              + sum_{f,j} bond[a,f,j] * filters[o,f,64+j]

Sharding: data-parallel over the 32 samples -> 4 samples per core x 8 cores.

DMA layout trick: the PE contracts over the partition dim, so conn must land
with n on partitions; conn's HBM layout is [a, n, f] with f (12 floats = 48 B)
innermost.  Splitting n = 4*nh + nl and loading tiles
[nh=128 partitions, (a, nl, f)] gives 192-byte contiguous runs per (partition,
atom) - 4x larger DMA descriptors than the naive n-on-partitions load - while
keeping the full contraction reachable via 4 accumulating matmuls (one per nl)
with W's rows permuted to match (computed on chip, so its layout is free).

Walrus quirk: a Matmult instruction (its LDWEIGHTS struct) can carry at most
ONE semaphore wait; Tile freely attaches several.  After Tile scheduling we
hoist the extra waits onto NoOps inserted directly before the matmul on the
same engine queue - semantically identical (waits execute in queue order).
"""

import sys

import numpy as np

try:
    import concourse.bass as bass
except ImportError:  # pragma: no cover
    sys.path.append("/opt/trn_rl_repo")
    import concourse.bass as bass

import concourse.mybir as mybir
from concourse import bass_utils
from concourse.tile import TileContext

N_SAMPLES, N_ATOMS = 32, 512
IN_DEPTH, OUT_DEPTH, FL = 64, 64, 12
N_CORES = 8
S_PER_CORE = N_SAMPLES // N_CORES  # 4

NL = 4  # low bits of n folded into the free dim (192-byte DMA runs)
NH = N_ATOMS // NL  # 128 partitions
A_BLK = 256  # atoms per connectivity tile
N_ABLK = N_ATOMS // A_BLK

PSW_BUFS = 4  # psum banks for W building
PO_BUFS = 4  # psum banks for output accumulation

_DT = mybir.dt.float32


def _build_bass(repeat=1, a_blk=A_BLK, c_bufs=2, dma_engines=("sync",),
                merge_ap=False, dma_parts=1, loop_repeat=1):
    """repeat > 1 re-runs phase 2 (the C stream + matmuls) that many times
    inside the NEFF - output is identical; used only to amortize the host
    dispatch overhead when measuring device-side time.

    a_blk: atoms per connectivity tile; c_bufs: tile-pool bufs for them.
    dma_engines: rotation of descriptor-generation engines for the C stream
    ("sync" and "scalar" are the two independent HWDGE rings, "gpsimd" is the
    SWDGE Q7 path).  dma_parts: split each tile's DMA into this many
    dma_starts over disjoint atom ranges, rotating engines.
    merge_ap: use the 3D [nh, a, (nl f)] access pattern instead of 4D."""
    nc = bass.Bass()
    c = nc.dram_tensor(
        "c", (S_PER_CORE, N_ATOMS, N_ATOMS, FL), _DT, kind="ExternalInput"
    )
    xt = nc.dram_tensor(
        "xt", (S_PER_CORE, IN_DEPTH, N_ATOMS), _DT, kind="ExternalInput"
    )
    bt = nc.dram_tensor("bt", (S_PER_CORE, 2 * FL, N_ATOMS), _DT, kind="ExternalInput")
    ftf = nc.dram_tensor("ftf", (IN_DEPTH, FL * OUT_DEPTH), _DT, kind="ExternalInput")
    fb = nc.dram_tensor("fb", (2 * FL, OUT_DEPTH), _DT, kind="ExternalInput")
    out_t = nc.dram_tensor(
        "out_t", (S_PER_CORE, OUT_DEPTH, N_ATOMS), _DT, kind="ExternalOutput"
    )

    HALF = FL * OUT_DEPTH // 2  # 384 columns per W-build matmul (one psum bank)

    with TileContext(nc) as tc:
        with (
            tc.tile_pool(name="consts", bufs=1) as consts,
            tc.tile_pool(name="cpool", bufs=c_bufs) as cpool,
            tc.tile_pool(name="wpool", bufs=S_PER_CORE) as wpool,
            tc.tile_pool(name="small", bufs=S_PER_CORE) as small,
            tc.tile_pool(name="outp", bufs=3) as outp,
            tc.tile_pool(name="psum", bufs=PO_BUFS, space="PSUM") as psum,
            tc.tile_pool(name="psumw", bufs=PSW_BUFS, space="PSUM") as psumw,
        ):
            ftf_sb = consts.tile([IN_DEPTH, FL * OUT_DEPTH], _DT)
            ftf_dma = nc.sync.dma_start(out=ftf_sb[:], in_=ftf[:])
            fb_sb = consts.tile([2 * FL, OUT_DEPTH], _DT)
            fb_dma = nc.sync.dma_start(out=fb_sb[:], in_=fb[:])

            xt_sbs, bt_sbs, xt_dmas, bt_dmas = [], [], [], []
            for s in range(S_PER_CORE):
                xt_sb = small.tile([IN_DEPTH, N_ATOMS], _DT, tag="xt")
                xt_dmas.append(nc.sync.dma_start(out=xt_sb[:], in_=xt[s]))
                xt_sbs.append(xt_sb)
                bt_sb = small.tile([2 * FL, N_ATOMS], _DT, tag="bt")
                bt_dmas.append(nc.sync.dma_start(out=bt_sb[:], in_=bt[s]))
                bt_sbs.append(bt_sb)

            # ---- Phase 1: W for all samples --------------------------------
            # W[nh, nl, f, o] = sum_d node[4*nh + nl, d] * filters[o, f, d]
            w_sbs = []
            for s in range(S_PER_CORE):
                w_sb = wpool.tile([NH, NL, FL, OUT_DEPTH], _DT, tag="w")
                w_sbs.append(w_sb)
                for j in range(NL):
                    for h in range(2):
                        pw = psumw.tile([NH, FL // 2, OUT_DEPTH], _DT, tag="pw")
                        nc.tensor.matmul(
                            pw[:],
                            lhsT=xt_sbs[s][:, j::NL],  # cols are n = 4*nh+j
                            rhs=ftf_sb[:, h * HALF : (h + 1) * HALF],
                            start=True,
                            stop=True,
                        )
                        nc.vector.tensor_copy(
                            out=w_sb[:, j, h * (FL // 2) : (h + 1) * (FL // 2), :],
                            in_=pw[:],
                        )

            # ---- Phase 2: stream connectivity, accumulate output -----------
            n_ablk = N_ATOMS // a_blk
            eng_map = {
                "sync": nc.sync,
                "scalar": nc.scalar,
                "gpsimd": nc.gpsimd,
            }
            dma_rot = 0

            import contextlib

            loop_ctx = (
                tc.For_i(0, loop_repeat, 1)
                if loop_repeat > 1
                else contextlib.nullcontext()
            )
            with loop_ctx:
                phase2(
                    nc, tc, repeat, a_blk, n_ablk, eng_map, dma_rot, merge_ap,
                    dma_parts, dma_engines, cpool, psum, outp, c, out_t,
                    fb_sb, bt_sbs, w_sbs,
                )

    _hoist_extra_waits(nc)
    return nc


def phase2(nc, tc, repeat, a_blk, n_ablk, eng_map, dma_rot, merge_ap,
           dma_parts, dma_engines, cpool, psum, outp, c, out_t,
           fb_sb, bt_sbs, w_sbs):
    if True:
        if True:
            for s in [s for _ in range(repeat) for s in range(S_PER_CORE)]:
                for ab in range(n_ablk):
                    a0 = ab * a_blk
                    if merge_ap:
                        ct = cpool.tile([NH, a_blk, NL * FL], _DT, tag="ct")
                        in_full = c[s, a0 : a0 + a_blk].rearrange(
                            "a (nh nl) f -> nh a (nl f)", nl=NL
                        )
                    else:
                        ct = cpool.tile([NH, a_blk, NL, FL], _DT, tag="ct")
                        in_full = c[s, a0 : a0 + a_blk].rearrange(
                            "a (nh nl) f -> nh a nl f", nl=NL
                        )
                    part = a_blk // dma_parts
                    for pi in range(dma_parts):
                        eng = eng_map[dma_engines[dma_rot % len(dma_engines)]]
                        dma_rot += 1
                        eng.dma_start(
                            out=ct[:, pi * part : (pi + 1) * part],
                            in_=in_full[:, pi * part : (pi + 1) * part],
                        )

                    po = psum.tile([OUT_DEPTH, a_blk], _DT, tag="po")
                    # bond contribution first: out[o,a] += fb[fj,o]^T @ bt[fj,a]
                    nc.tensor.matmul(
                        po[:],
                        lhsT=fb_sb[:],
                        rhs=bt_sbs[s][:, a0 : a0 + a_blk],
                        start=True,
                        stop=False,
                    )
                    for f in range(FL):
                        for j in range(NL):
                            rhs = (
                                ct[:, :, j * FL + f]
                                if merge_ap
                                else ct[:, :, j, f]
                            )
                            nc.tensor.matmul(
                                po[:],
                                lhsT=w_sbs[s][:, j, f, :],  # [128, 64]
                                rhs=rhs,  # [128, a_blk]
                                start=False,
                                stop=(f == FL - 1 and j == NL - 1),
                            )
                    ot = outp.tile([OUT_DEPTH, a_blk], _DT, tag="ot")
                    nc.vector.tensor_copy(out=ot[:], in_=po[:])
                    nc.sync.dma_start(
                        out=out_t[s, :, a0 : a0 + a_blk], in_=ot[:]
                    )


NL2 = 16  # v2: low bits of n in the free dim -> 768-byte runs (no RMW penalty)
NH2 = N_ATOMS // NL2  # 32 partitions per sample


def _build_bass_v2(a_blk=64, c_bufs=2, po_bufs=6, loop_repeat=1):
    """v2: 768-byte DMA runs via n = 16*nh + nl, with the 4 samples packed
    across the 4 partition quarters (p = 32*s + nh).  The four 32-partition
    DMAs per atom chunk cover complementary engine sets of the SBUF port
    swizzle, and the K=32 matmuls use tile_position row groups (4 concurrent)
    with each sample's W stored in its partition quarter of one W tile."""
    nc = bass.Bass()
    c = nc.dram_tensor(
        "c", (S_PER_CORE, N_ATOMS, N_ATOMS, FL), _DT, kind="ExternalInput"
    )
    xt = nc.dram_tensor(
        "xt", (S_PER_CORE, IN_DEPTH, N_ATOMS), _DT, kind="ExternalInput"
    )
    bt = nc.dram_tensor("bt", (S_PER_CORE, 2 * FL, N_ATOMS), _DT, kind="ExternalInput")
    ftf = nc.dram_tensor("ftf", (IN_DEPTH, FL * OUT_DEPTH), _DT, kind="ExternalInput")
    fb = nc.dram_tensor("fb", (2 * FL, OUT_DEPTH), _DT, kind="ExternalInput")
    out_t = nc.dram_tensor(
        "out_t", (S_PER_CORE, OUT_DEPTH, N_ATOMS), _DT, kind="ExternalOutput"
    )

    HALF = FL * OUT_DEPTH // 2  # 384
    n_ablk = N_ATOMS // a_blk

    with TileContext(nc) as tc:
        with (
            tc.tile_pool(name="consts", bufs=1) as consts,
            tc.tile_pool(name="cpool", bufs=c_bufs) as cpool,
            tc.tile_pool(name="wpool", bufs=1) as wpool,
            tc.tile_pool(name="small", bufs=S_PER_CORE) as small,
            tc.tile_pool(name="outp", bufs=4) as outp,
            tc.tile_pool(name="psum", bufs=po_bufs, space="PSUM") as psum,
            tc.tile_pool(name="psumw", bufs=2, space="PSUM") as psumw,
        ):
            ftf_sb = consts.tile([IN_DEPTH, FL * OUT_DEPTH], _DT)
            nc.sync.dma_start(out=ftf_sb[:], in_=ftf[:])
            fb_sb = consts.tile([2 * FL, OUT_DEPTH], _DT)
            nc.sync.dma_start(out=fb_sb[:], in_=fb[:])

            xt_sbs, bt_sbs = [], []
            for s in range(S_PER_CORE):
                xt_sb = small.tile([IN_DEPTH, N_ATOMS], _DT, tag="xt")
                nc.sync.dma_start(out=xt_sb[:], in_=xt[s])
                xt_sbs.append(xt_sb)
                bt_sb = small.tile([2 * FL, N_ATOMS], _DT, tag="bt")
                nc.sync.dma_start(out=bt_sb[:], in_=bt[s])
                bt_sbs.append(bt_sb)

            # ---- Phase 1: W4[p=32s+nh, j, f, o] = W_s[n=16*nh+j, f, o] ------
            w4 = wpool.tile([128, NL2, FL, OUT_DEPTH], _DT)
            for j in range(NL2):
                for h in range(2):
                    pw = psumw.tile([128, FL // 2, OUT_DEPTH], _DT, tag="pw")
                    for s in range(S_PER_CORE):
                        nc.tensor.matmul(
                            pw[32 * s : 32 * s + 32],
                            lhsT=xt_sbs[s][:, j::NL2],  # [64, 32] cols n=16nh+j
                            rhs=ftf_sb[:, h * HALF : (h + 1) * HALF],
                            start=True,
                            stop=True,
                            tile_position=(0, 32 * s),
                        )
                    nc.vector.tensor_copy(
                        out=w4[:, j, h * (FL // 2) : (h + 1) * (FL // 2), :],
                        in_=pw[:],
                    )

            # ---- Phase 2: stream connectivity, accumulate output -----------
            import contextlib

            loop_ctx = (
                tc.For_i(0, loop_repeat, 1)
                if loop_repeat > 1
                else contextlib.nullcontext()
            )
            with loop_ctx:
                for ab in range(n_ablk):
                    a0 = ab * a_blk
                    ct = cpool.tile([128, a_blk, NL2 * FL], _DT, tag="ct")
                    for s in range(S_PER_CORE):
                        nc.sync.dma_start(
                            out=ct[32 * s : 32 * s + 32],
                            in_=c[s, a0 : a0 + a_blk].rearrange(
                                "a (nh nl) f -> nh a (nl f)", nl=NL2
                            ),
                        )
                    pos = []
                    for s in range(S_PER_CORE):
                        po = psum.tile([OUT_DEPTH, a_blk], _DT, tag="po")
                        pos.append(po)
                        nc.tensor.matmul(
                            po[:],
                            lhsT=fb_sb[:],
                            rhs=bt_sbs[s][:, a0 : a0 + a_blk],
                            start=True,
                            stop=False,
                            tile_position=(0, 0),
                        )
                    for f in range(FL):
                        for j in range(NL2):
                            for s in range(S_PER_CORE):
                                nc.tensor.matmul(
                                    pos[s][:],
                                    lhsT=w4[32 * s : 32 * s + 32, j, f, :],
                                    rhs=ct[32 * s : 32 * s + 32, :, j * FL + f],
                                    start=False,
                                    stop=(f == FL - 1 and j == NL2 - 1),
                                    tile_position=(32 * s, 0),
                                )
                    for s in range(S_PER_CORE):
                        ot = outp.tile([OUT_DEPTH, a_blk], _DT, tag="ot")
                        nc.vector.tensor_copy(out=ot[:], in_=pos[s][:])
                        nc.sync.dma_start(
                            out=out_t[s, :, a0 : a0 + a_blk], in_=ot[:]
                        )

    _hoist_extra_waits(nc)
    return nc


def _build_bass_v3(c_bufs=1, po_bufs=6, loop_repeat=1):
    """v3: 768-byte DMA runs (n = 16*nh + nl) with the partition quarters
    holding the four 128-atom chunks of ONE sample (p = 32*q + nh).  Matmuls
    are K=32 at tile_position row group 32*q with N=128, 4-way concurrent;
    per-sample W is built once at quarter 0 and replicated to the other
    quarters by SBUF->SBUF DMA (cross-partition copies are DMA-only)."""
    nc = bass.Bass()
    c = nc.dram_tensor(
        "c", (S_PER_CORE, N_ATOMS, N_ATOMS, FL), _DT, kind="ExternalInput"
    )
    xt = nc.dram_tensor(
        "xt", (S_PER_CORE, IN_DEPTH, N_ATOMS), _DT, kind="ExternalInput"
    )
    bt = nc.dram_tensor("bt", (S_PER_CORE, 2 * FL, N_ATOMS), _DT, kind="ExternalInput")
    ftf = nc.dram_tensor("ftf", (IN_DEPTH, FL * OUT_DEPTH), _DT, kind="ExternalInput")
    fb = nc.dram_tensor("fb", (2 * FL, OUT_DEPTH), _DT, kind="ExternalInput")
    out_t = nc.dram_tensor(
        "out_t", (S_PER_CORE, OUT_DEPTH, N_ATOMS), _DT, kind="ExternalOutput"
    )

    HALF = FL * OUT_DEPTH // 2  # 384
    AQ = N_ATOMS // 4  # 128 atoms per partition quarter

    with TileContext(nc) as tc:
        with (
            tc.tile_pool(name="consts", bufs=1) as consts,
            tc.tile_pool(name="cpool", bufs=c_bufs) as cpool,
            tc.tile_pool(name="wpool", bufs=1) as wpool,
            tc.tile_pool(name="small", bufs=S_PER_CORE) as small,
            tc.tile_pool(name="outp", bufs=4) as outp,
            tc.tile_pool(name="psum", bufs=po_bufs, space="PSUM") as psum,
            tc.tile_pool(name="psumw", bufs=2, space="PSUM") as psumw,
        ):
            ftf_sb = consts.tile([IN_DEPTH, FL * OUT_DEPTH], _DT)
            nc.sync.dma_start(out=ftf_sb[:], in_=ftf[:])
            fb_sb = consts.tile([2 * FL, OUT_DEPTH], _DT)
            nc.sync.dma_start(out=fb_sb[:], in_=fb[:])

            xt_sbs, bt_sbs = [], []
            for s in range(S_PER_CORE):
                xt_sb = small.tile([IN_DEPTH, N_ATOMS], _DT, tag="xt")
                nc.sync.dma_start(out=xt_sb[:], in_=xt[s])
                xt_sbs.append(xt_sb)
                bt_sb = small.tile([2 * FL, N_ATOMS], _DT, tag="bt")
                nc.sync.dma_start(out=bt_sb[:], in_=bt[s])
                bt_sbs.append(bt_sb)

            import contextlib

            loop_ctx = (
                tc.For_i(0, loop_repeat, 1)
                if loop_repeat > 1
                else contextlib.nullcontext()
            )
            with loop_ctx:
                for s in range(S_PER_CORE):
                    # ---- W_s[n=16*nh+j, f, o] at quarter 0, then replicate --
                    w4 = wpool.tile([128, NL2, FL, OUT_DEPTH], _DT, tag="w4")
                    for j in range(NL2):
                        for h in range(2):
                            pw = psumw.tile([NH2, FL // 2, OUT_DEPTH], _DT, tag="pw")
                            nc.tensor.matmul(
                                pw[:],
                                lhsT=xt_sbs[s][:, j::NL2],  # [64, 32]
                                rhs=ftf_sb[:, h * HALF : (h + 1) * HALF],
                                start=True,
                                stop=True,
                                tile_position=(0, 0),
                            )
                            nc.vector.tensor_copy(
                                out=w4[:NH2, j, h * (FL // 2) : (h + 1) * (FL // 2), :],
                                in_=pw[:],
                            )
                    for q in range(1, 4):
                        nc.sync.dma_start(
                            out=w4[32 * q : 32 * q + 32], in_=w4[0:32]
                        )

                    # ---- C stream: quarter q holds atoms [128q, 128q+128) --
                    ct = cpool.tile([128, AQ, NL2 * FL], _DT, tag="ct")
                    for q in range(4):
                        nc.sync.dma_start(
                            out=ct[32 * q : 32 * q + 32],
                            in_=c[s, AQ * q : AQ * (q + 1)].rearrange(
                                "a (nh nl) f -> nh a (nl f)", nl=NL2
                            ),
                        )
                    pos = []
                    for q in range(4):
                        po = psum.tile([OUT_DEPTH, AQ], _DT, tag="po")
                        pos.append(po)
                        nc.tensor.matmul(
                            po[:],
                            lhsT=fb_sb[:],
                            rhs=bt_sbs[s][:, AQ * q : AQ * (q + 1)],
                            start=True,
                            stop=False,
                            tile_position=(0, 0),
                        )
                    for f in range(FL):
                        for j in range(NL2):
                            for q in range(4):
                                nc.tensor.matmul(
                                    pos[q][:],
                                    lhsT=w4[32 * q : 32 * q + 32, j, f, :],
                                    rhs=ct[32 * q : 32 * q + 32, :, j * FL + f],
                                    start=False,
                                    stop=(f == FL - 1 and j == NL2 - 1),
                                    tile_position=(32 * q, 0),
                                )
                    for q in range(4):
                        ot = outp.tile([OUT_DEPTH, AQ], _DT, tag="ot")
                        nc.vector.tensor_copy(out=ot[:], in_=pos[q][:])
                        nc.sync.dma_start(
                            out=out_t[s, :, AQ * q : AQ * (q + 1)], in_=ot[:]
                        )

    _hoist_extra_waits(nc)
    return nc


def _build_bass_v4(a_blk=128, c_bufs=3, c2_bufs=2, po_bufs=4, loop_repeat=1, dma_split=True):
    """v4: strided DMA load [nh, a, nl, f] (192-B runs), then an on-chip
    free-dim permute to [nh, nl, f, a] split across DVE and ACT (engines read
    strided at full rate; the PE's moving-operand fetch does not - a strided
    rhs streams ~4x slower, measured).  Matmuls then read contiguous [128, a]
    slices, split-K across the two 64-row col groups of the PE (concurrent),
    with a final DVE add of the two halves."""
    nc = bass.Bass()
    c = nc.dram_tensor(
        "c", (S_PER_CORE, N_ATOMS, N_ATOMS, FL), _DT, kind="ExternalInput"
    )
    xt = nc.dram_tensor(
        "xt", (S_PER_CORE, IN_DEPTH, N_ATOMS), _DT, kind="ExternalInput"
    )
    bt = nc.dram_tensor("bt", (S_PER_CORE, 2 * FL, N_ATOMS), _DT, kind="ExternalInput")
    ftf = nc.dram_tensor("ftf", (IN_DEPTH, FL * OUT_DEPTH), _DT, kind="ExternalInput")
    fb = nc.dram_tensor("fb", (2 * FL, OUT_DEPTH), _DT, kind="ExternalInput")
    out_t = nc.dram_tensor(
        "out_t", (S_PER_CORE, OUT_DEPTH, N_ATOMS), _DT, kind="ExternalOutput"
    )

    HALF = FL * OUT_DEPTH // 2  # 384
    n_ablk = N_ATOMS // a_blk
    KCH = FL * NL  # 48 contraction chunks of K=128

    with TileContext(nc) as tc:
        with (
            tc.tile_pool(name="consts", bufs=1) as consts,
            tc.tile_pool(name="cpool", bufs=c_bufs) as cpool,
            tc.tile_pool(name="c2pool", bufs=c2_bufs) as c2pool,
            tc.tile_pool(name="wpool", bufs=S_PER_CORE) as wpool,
            tc.tile_pool(name="small", bufs=S_PER_CORE) as small,
            tc.tile_pool(name="outp", bufs=4) as outp,
            tc.tile_pool(name="psum", bufs=po_bufs, space="PSUM") as psum,
            tc.tile_pool(name="psumw", bufs=2, space="PSUM") as psumw,
        ):
            ftf_sb = consts.tile([IN_DEPTH, FL * OUT_DEPTH], _DT)
            nc.sync.dma_start(out=ftf_sb[:], in_=ftf[:])
            fb_sb = consts.tile([2 * FL, OUT_DEPTH], _DT)
            nc.sync.dma_start(out=fb_sb[:], in_=fb[:])

            xt_sbs, bt_sbs = [], []
            for s in range(S_PER_CORE):
                xt_sb = small.tile([IN_DEPTH, N_ATOMS], _DT, tag="xt")
                nc.sync.dma_start(out=xt_sb[:], in_=xt[s])
                xt_sbs.append(xt_sb)
                bt_sb = small.tile([2 * FL, N_ATOMS], _DT, tag="bt")
                nc.sync.dma_start(out=bt_sb[:], in_=bt[s])
                bt_sbs.append(bt_sb)

            # ---- Phase 1: W[nh, j, f, o] for all samples -------------------
            w_sbs = []
            for s in range(S_PER_CORE):
                w_sb = wpool.tile([NH, NL, FL, OUT_DEPTH], _DT, tag="w")
                w_sbs.append(w_sb)
                for j in range(NL):
                    for h in range(2):
                        pw = psumw.tile([NH, FL // 2, OUT_DEPTH], _DT, tag="pw")
                        nc.tensor.matmul(
                            pw[:],
                            lhsT=xt_sbs[s][:, j::NL],
                            rhs=ftf_sb[:, h * HALF : (h + 1) * HALF],
                            start=True,
                            stop=True,
                        )
                        nc.vector.tensor_copy(
                            out=w_sb[:, j, h * (FL // 2) : (h + 1) * (FL // 2), :],
                            in_=pw[:],
                        )

            # ---- Phase 2 ----------------------------------------------------
            import contextlib

            loop_ctx = (
                tc.For_i(0, loop_repeat, 1)
                if loop_repeat > 1
                else contextlib.nullcontext()
            )
            with loop_ctx:
                for s in range(S_PER_CORE):
                    for ab in range(n_ablk):
                        a0 = ab * a_blk
                        ct = cpool.tile([NH, a_blk, NL, FL], _DT, tag="ct")
                        cin = c[s, a0 : a0 + a_blk].rearrange(
                            "a (nh nl) f -> nh a nl f", nl=NL
                        )
                        if dma_split:
                            hh = a_blk // 2
                            nc.sync.dma_start(out=ct[:, :hh], in_=cin[:, :hh])
                            nc.scalar.dma_start(out=ct[:, hh:], in_=cin[:, hh:])
                        else:
                            nc.sync.dma_start(out=ct[:], in_=cin)
                        # permute (a, nl, f) -> (nl, f, a); DVE 2/3, ACT 1/3
                        ct2 = c2pool.tile([NH, NL, FL, a_blk], _DT, tag="ct2")
                        nc.vector.tensor_copy(
                            out=ct2[:, 0:3],
                            in_=ct[:, :, 0:3].rearrange("p a j f -> p j f a"),
                        )
                        nc.scalar.copy(
                            out=ct2[:, 3:4],
                            in_=ct[:, :, 3:4].rearrange("p a j f -> p j f a"),
                        )

                        po = psum.tile([OUT_DEPTH, a_blk], _DT, tag="po")
                        nc.tensor.matmul(
                            po[:],
                            lhsT=fb_sb[:],
                            rhs=bt_sbs[s][:, a0 : a0 + a_blk],
                            start=True,
                            stop=False,
                        )
                        for k in range(KCH):
                            j, f = k // FL, k % FL
                            nc.tensor.matmul(
                                po[:],
                                lhsT=w_sbs[s][:, j, f, :],
                                rhs=ct2[:, j, f, :],
                                start=False,
                                stop=(k == KCH - 1),
                            )
                        ot = outp.tile([OUT_DEPTH, a_blk], _DT, tag="ot")
                        nc.vector.tensor_copy(out=ot[:], in_=po[:])
                        nc.scalar.dma_start(
                            out=out_t[s, :, a0 : a0 + a_blk], in_=ot[:]
                        )

    _hoist_extra_waits(nc)
    return nc


def _hoist_extra_waits(nc):
    """This walrus build rejects any instruction struct carrying more than one
    semaphore wait ("Too many sync wait commands"); Tile freely attaches
    several.  Waits execute in engine-queue order, so hoisting the extras onto
    NoOps inserted directly before the instruction on the same engine is
    semantically identical.  One wait per NoOp."""
    for f in nc.m.functions:
        for blk in f.blocks:
            insts = blk.instructions
            rebuilt = []
            changed = False
            for inst in insts:
                si = inst.sync_info
                if si is not None and len(si.on_wait) > 1:
                    for w in list(si.on_wait)[:-1]:
                        nop = mybir.InstNoOp(
                            name=nc.get_next_instruction_name(),
                            engine=inst.engine,
                            ins=[],
                            outs=[],
                            sync_info=mybir.SyncInfo(on_wait=[w], on_update=[]),
                        )
                        nc.register_instruction(nop)
                        rebuilt.append(nop)
                    inst.sync_info = mybir.SyncInfo(
                        on_wait=[list(si.on_wait)[-1]], on_update=list(si.on_update)
                    )
                    changed = True
                rebuilt.append(inst)
            if changed:
                del insts[:]
                insts.extend(rebuilt)


_CACHED_NC = {}


def _get_nc(repeat=1, version="v1", **kw):
    key = (repeat, version, tuple(sorted(kw.items())))
    if key not in _CACHED_NC:
        if version == "v2":
            _CACHED_NC[key] = _build_bass_v2(**kw)
        elif version == "v3":
            _CACHED_NC[key] = _build_bass_v3(**kw)
        elif version == "v4":
            _CACHED_NC[key] = _build_bass_v4(**kw)
        else:
            _CACHED_NC[key] = _build_bass(repeat, **kw)
    return _CACHED_NC[key]


def _prep_inputs(node, conn, bond, filters):
    """Host-side layout-only prep + per-core sharding."""
    node = np.asarray(node, dtype=np.float32)
    conn = np.asarray(conn, dtype=np.float32)
    bond = np.asarray(bond, dtype=np.float32)
    filters = np.asarray(filters, dtype=np.float32)

    # xt[s, d, n] = node[s, n, d]
    node_t = np.ascontiguousarray(node.transpose(0, 2, 1))
    # bt[s, f*2+j, a] = bond[s, a, f, j]
    bond_t = np.ascontiguousarray(bond.transpose(0, 2, 3, 1)).reshape(
        N_SAMPLES, 2 * FL, N_ATOMS
    )
    # ftf[d, f*64+o] = filters[o, f, d]
    ftf = np.ascontiguousarray(filters[:, :, :IN_DEPTH].transpose(2, 1, 0)).reshape(
        IN_DEPTH, FL * OUT_DEPTH
    )
    # fb[f*2+j, o] = filters[o, f, 64+j]
    fb = np.ascontiguousarray(filters[:, :, IN_DEPTH:].transpose(1, 2, 0)).reshape(
        2 * FL, OUT_DEPTH
    )

    in_maps = []
    for k in range(N_CORES):
        s0 = k * S_PER_CORE
        in_maps.append(
            {
                "c": np.ascontiguousarray(conn[s0 : s0 + S_PER_CORE]),
                "xt": np.ascontiguousarray(node_t[s0 : s0 + S_PER_CORE]),
                "bt": np.ascontiguousarray(bond_t[s0 : s0 + S_PER_CORE]),
                "ftf": ftf,
                "fb": fb,
            }
        )
    return in_maps


# Best-measured variant used by kernel(); see bench logs in the session.
_BEST_VERSION = "v4"


def run(node_property_tensor, connectivity_tensor, bond_property_tensor, filters,
        trace=False, version=None):
    """Run on 8 cores; returns (output [32,512,64], BassKernelResults)."""
    nc = _get_nc(version=version or _BEST_VERSION)
    in_maps = _prep_inputs(
        node_property_tensor, connectivity_tensor, bond_property_tensor, filters
    )
    res = bass_utils.run_bass_kernel_spmd(
        nc, in_maps, core_ids=list(range(N_CORES)), trace=trace
    )
    out = np.empty((N_SAMPLES, N_ATOMS, OUT_DEPTH), dtype=np.float32)
    for k in range(N_CORES):
        s0 = k * S_PER_CORE
        out[s0 : s0 + S_PER_CORE] = res.results[k]["out_t"].transpose(0, 2, 1)
    return out, res


def kernel(node_property_tensor, connectivity_tensor, bond_property_tensor, filters):
    out, _ = run(
        node_property_tensor, connectivity_tensor, bond_property_tensor, filters
    )
    return out



# revision 17
# speedup vs baseline: 26.5406x; 26.5406x over previous
"""ChemConv (GNN message passing) kernel for Trainium2, 8 NeuronCores.

Reference math (per sample s):
    node_conn[a,f,d] = sum_n conn[a,n,f] * node[n,d]
    out[a,o]         = sum_{f,d} cat(node_conn, bond)[a,f,d] * filters[o,f,d]

Folded form used on device (filters folded into node features):
    W[n,f,o]  = sum_d node[n,d] * filters[o,f,d]          (tiny matmuls)
    out[a,o]  = sum_{n,f} conn[a,(n,f)] * W[(n,f),o]
              + sum_{f,j} bond[a,f,j] * filters[o,f,64+j]

Sharding: data-parallel over the 32 samples -> 4 samples per core x 8 cores.

Best variant (v6): the kernel is HBM-bound on streaming conn (the only big
tensor), so the host pre-arranges conn into exactly the tiles the PE wants:
cb[s, p, c, a] bf16, where chunk c of contraction rows [128c, 128c+128) over
K = (f, n) maps to (f = c//4, n-block = c%4).  Each partition's (c, a) block
is one contiguous HBM run, so conn streams at line rate (~318 GB/s measured
under 8-core load) with no on-chip permute, and the bf16 cast halves HBM
bytes (rel err ~2e-3, gate 2e-2).  Matmuls are then plain K=128 accumulating
GEMMs with 512-wide bf16 moving data (1 cycle/row; fp32 is 4).  W tiles
[128 (n), 12 (f), 64 (o)] bf16 are built on chip once per call (phase 1) to
match the chunk order.  Conn loads ride one HWDGE ring (sync) in half-sample
tiles x6 bufs so the load stream never stalls; output stores ride the other
ring (scalar).  Older variants (v1-v5) kept for reference / probes.

Walrus quirk: a Matmult instruction (its LDWEIGHTS struct) can carry at most
ONE semaphore wait; Tile freely attaches several.  After Tile scheduling we
hoist the extra waits onto NoOps inserted directly before the matmul on the
same engine queue - semantically identical (waits execute in queue order).
"""

import sys

import numpy as np

try:
    import concourse.bass as bass
except ImportError:  # pragma: no cover
    sys.path.append("/opt/trn_rl_repo")
    import concourse.bass as bass

import concourse.mybir as mybir
from concourse import bass_utils
from concourse.tile import TileContext

N_SAMPLES, N_ATOMS = 32, 512
IN_DEPTH, OUT_DEPTH, FL = 64, 64, 12
N_CORES = 8
S_PER_CORE = N_SAMPLES // N_CORES  # 4

NL = 4  # low bits of n folded into the free dim (192-byte DMA runs)
NH = N_ATOMS // NL  # 128 partitions
A_BLK = 256  # atoms per connectivity tile
N_ABLK = N_ATOMS // A_BLK

PSW_BUFS = 4  # psum banks for W building
PO_BUFS = 4  # psum banks for output accumulation

_DT = mybir.dt.float32


def _build_bass(repeat=1, a_blk=A_BLK, c_bufs=2, dma_engines=("sync",),
                merge_ap=False, dma_parts=1, loop_repeat=1):
    """repeat > 1 re-runs phase 2 (the C stream + matmuls) that many times
    inside the NEFF - output is identical; used only to amortize the host
    dispatch overhead when measuring device-side time.

    a_blk: atoms per connectivity tile; c_bufs: tile-pool bufs for them.
    dma_engines: rotation of descriptor-generation engines for the C stream
    ("sync" and "scalar" are the two independent HWDGE rings, "gpsimd" is the
    SWDGE Q7 path).  dma_parts: split each tile's DMA into this many
    dma_starts over disjoint atom ranges, rotating engines.
    merge_ap: use the 3D [nh, a, (nl f)] access pattern instead of 4D."""
    nc = bass.Bass()
    c = nc.dram_tensor(
        "c", (S_PER_CORE, N_ATOMS, N_ATOMS, FL), _DT, kind="ExternalInput"
    )
    xt = nc.dram_tensor(
        "xt", (S_PER_CORE, IN_DEPTH, N_ATOMS), _DT, kind="ExternalInput"
    )
    bt = nc.dram_tensor("bt", (S_PER_CORE, 2 * FL, N_ATOMS), _DT, kind="ExternalInput")
    ftf = nc.dram_tensor("ftf", (IN_DEPTH, FL * OUT_DEPTH), _DT, kind="ExternalInput")
    fb = nc.dram_tensor("fb", (2 * FL, OUT_DEPTH), _DT, kind="ExternalInput")
    out_t = nc.dram_tensor(
        "out_t", (S_PER_CORE, OUT_DEPTH, N_ATOMS), _DT, kind="ExternalOutput"
    )

    HALF = FL * OUT_DEPTH // 2  # 384 columns per W-build matmul (one psum bank)

    with TileContext(nc) as tc:
        with (
            tc.tile_pool(name="consts", bufs=1) as consts,
            tc.tile_pool(name="cpool", bufs=c_bufs) as cpool,
            tc.tile_pool(name="wpool", bufs=S_PER_CORE) as wpool,
            tc.tile_pool(name="small", bufs=S_PER_CORE) as small,
            tc.tile_pool(name="outp", bufs=3) as outp,
            tc.tile_pool(name="psum", bufs=PO_BUFS, space="PSUM") as psum,
            tc.tile_pool(name="psumw", bufs=PSW_BUFS, space="PSUM") as psumw,
        ):
            ftf_sb = consts.tile([IN_DEPTH, FL * OUT_DEPTH], _DT)
            ftf_dma = nc.sync.dma_start(out=ftf_sb[:], in_=ftf[:])
            fb_sb = consts.tile([2 * FL, OUT_DEPTH], _DT)
            fb_dma = nc.sync.dma_start(out=fb_sb[:], in_=fb[:])

            xt_sbs, bt_sbs, xt_dmas, bt_dmas = [], [], [], []
            for s in range(S_PER_CORE):
                xt_sb = small.tile([IN_DEPTH, N_ATOMS], _DT, tag="xt")
                xt_dmas.append(nc.sync.dma_start(out=xt_sb[:], in_=xt[s]))
                xt_sbs.append(xt_sb)
                bt_sb = small.tile([2 * FL, N_ATOMS], _DT, tag="bt")
                bt_dmas.append(nc.sync.dma_start(out=bt_sb[:], in_=bt[s]))
                bt_sbs.append(bt_sb)

            # ---- Phase 1: W for all samples --------------------------------
            # W[nh, nl, f, o] = sum_d node[4*nh + nl, d] * filters[o, f, d]
            w_sbs = []
            for s in range(S_PER_CORE):
                w_sb = wpool.tile([NH, NL, FL, OUT_DEPTH], _DT, tag="w")
                w_sbs.append(w_sb)
                for j in range(NL):
                    for h in range(2):
                        pw = psumw.tile([NH, FL // 2, OUT_DEPTH], _DT, tag="pw")
                        nc.tensor.matmul(
                            pw[:],
                            lhsT=xt_sbs[s][:, j::NL],  # cols are n = 4*nh+j
                            rhs=ftf_sb[:, h * HALF : (h + 1) * HALF],
                            start=True,
                            stop=True,
                        )
                        nc.vector.tensor_copy(
                            out=w_sb[:, j, h * (FL // 2) : (h + 1) * (FL // 2), :],
                            in_=pw[:],
                        )

            # ---- Phase 2: stream connectivity, accumulate output -----------
            n_ablk = N_ATOMS // a_blk
            eng_map = {
                "sync": nc.sync,
                "scalar": nc.scalar,
                "gpsimd": nc.gpsimd,
            }
            dma_rot = 0

            import contextlib

            loop_ctx = (
                tc.For_i(0, loop_repeat, 1)
                if loop_repeat > 1
                else contextlib.nullcontext()
            )
            with loop_ctx:
                phase2(
                    nc, tc, repeat, a_blk, n_ablk, eng_map, dma_rot, merge_ap,
                    dma_parts, dma_engines, cpool, psum, outp, c, out_t,
                    fb_sb, bt_sbs, w_sbs,
                )

    _hoist_extra_waits(nc)
    return nc


def phase2(nc, tc, repeat, a_blk, n_ablk, eng_map, dma_rot, merge_ap,
           dma_parts, dma_engines, cpool, psum, outp, c, out_t,
           fb_sb, bt_sbs, w_sbs):
    if True:
        if True:
            for s in [s for _ in range(repeat) for s in range(S_PER_CORE)]:
                for ab in range(n_ablk):
                    a0 = ab * a_blk
                    if merge_ap:
                        ct = cpool.tile([NH, a_blk, NL * FL], _DT, tag="ct")
                        in_full = c[s, a0 : a0 + a_blk].rearrange(
                            "a (nh nl) f -> nh a (nl f)", nl=NL
                        )
                    else:
                        ct = cpool.tile([NH, a_blk, NL, FL], _DT, tag="ct")
                        in_full = c[s, a0 : a0 + a_blk].rearrange(
                            "a (nh nl) f -> nh a nl f", nl=NL
                        )
                    part = a_blk // dma_parts
                    for pi in range(dma_parts):
                        eng = eng_map[dma_engines[dma_rot % len(dma_engines)]]
                        dma_rot += 1
                        eng.dma_start(
                            out=ct[:, pi * part : (pi + 1) * part],
                            in_=in_full[:, pi * part : (pi + 1) * part],
                        )

                    po = psum.tile([OUT_DEPTH, a_blk], _DT, tag="po")
                    # bond contribution first: out[o,a] += fb[fj,o]^T @ bt[fj,a]
                    nc.tensor.matmul(
                        po[:],
                        lhsT=fb_sb[:],
                        rhs=bt_sbs[s][:, a0 : a0 + a_blk],
                        start=True,
                        stop=False,
                    )
                    for f in range(FL):
                        for j in range(NL):
                            rhs = (
                                ct[:, :, j * FL + f]
                                if merge_ap
                                else ct[:, :, j, f]
                            )
                            nc.tensor.matmul(
                                po[:],
                                lhsT=w_sbs[s][:, j, f, :],  # [128, 64]
                                rhs=rhs,  # [128, a_blk]
                                start=False,
                                stop=(f == FL - 1 and j == NL - 1),
                            )
                    ot = outp.tile([OUT_DEPTH, a_blk], _DT, tag="ot")
                    nc.vector.tensor_copy(out=ot[:], in_=po[:])
                    nc.sync.dma_start(
                        out=out_t[s, :, a0 : a0 + a_blk], in_=ot[:]
                    )


NL2 = 16  # v2: low bits of n in the free dim -> 768-byte runs (no RMW penalty)
NH2 = N_ATOMS // NL2  # 32 partitions per sample


def _build_bass_v2(a_blk=64, c_bufs=2, po_bufs=6, loop_repeat=1):
    """v2: 768-byte DMA runs via n = 16*nh + nl, with the 4 samples packed
    across the 4 partition quarters (p = 32*s + nh).  The four 32-partition
    DMAs per atom chunk cover complementary engine sets of the SBUF port
    swizzle, and the K=32 matmuls use tile_position row groups (4 concurrent)
    with each sample's W stored in its partition quarter of one W tile."""
    nc = bass.Bass()
    c = nc.dram_tensor(
        "c", (S_PER_CORE, N_ATOMS, N_ATOMS, FL), _DT, kind="ExternalInput"
    )
    xt = nc.dram_tensor(
        "xt", (S_PER_CORE, IN_DEPTH, N_ATOMS), _DT, kind="ExternalInput"
    )
    bt = nc.dram_tensor("bt", (S_PER_CORE, 2 * FL, N_ATOMS), _DT, kind="ExternalInput")
    ftf = nc.dram_tensor("ftf", (IN_DEPTH, FL * OUT_DEPTH), _DT, kind="ExternalInput")
    fb = nc.dram_tensor("fb", (2 * FL, OUT_DEPTH), _DT, kind="ExternalInput")
    out_t = nc.dram_tensor(
        "out_t", (S_PER_CORE, OUT_DEPTH, N_ATOMS), _DT, kind="ExternalOutput"
    )

    HALF = FL * OUT_DEPTH // 2  # 384
    n_ablk = N_ATOMS // a_blk

    with TileContext(nc) as tc:
        with (
            tc.tile_pool(name="consts", bufs=1) as consts,
            tc.tile_pool(name="cpool", bufs=c_bufs) as cpool,
            tc.tile_pool(name="wpool", bufs=1) as wpool,
            tc.tile_pool(name="small", bufs=S_PER_CORE) as small,
            tc.tile_pool(name="outp", bufs=4) as outp,
            tc.tile_pool(name="psum", bufs=po_bufs, space="PSUM") as psum,
            tc.tile_pool(name="psumw", bufs=2, space="PSUM") as psumw,
        ):
            ftf_sb = consts.tile([IN_DEPTH, FL * OUT_DEPTH], _DT)
            nc.sync.dma_start(out=ftf_sb[:], in_=ftf[:])
            fb_sb = consts.tile([2 * FL, OUT_DEPTH], _DT)
            nc.sync.dma_start(out=fb_sb[:], in_=fb[:])

            xt_sbs, bt_sbs = [], []
            for s in range(S_PER_CORE):
                xt_sb = small.tile([IN_DEPTH, N_ATOMS], _DT, tag="xt")
                nc.sync.dma_start(out=xt_sb[:], in_=xt[s])
                xt_sbs.append(xt_sb)
                bt_sb = small.tile([2 * FL, N_ATOMS], _DT, tag="bt")
                nc.sync.dma_start(out=bt_sb[:], in_=bt[s])
                bt_sbs.append(bt_sb)

            # ---- Phase 1: W4[p=32s+nh, j, f, o] = W_s[n=16*nh+j, f, o] ------
            w4 = wpool.tile([128, NL2, FL, OUT_DEPTH], _DT)
            for j in range(NL2):
                for h in range(2):
                    pw = psumw.tile([128, FL // 2, OUT_DEPTH], _DT, tag="pw")
                    for s in range(S_PER_CORE):
                        nc.tensor.matmul(
                            pw[32 * s : 32 * s + 32],
                            lhsT=xt_sbs[s][:, j::NL2],  # [64, 32] cols n=16nh+j
                            rhs=ftf_sb[:, h * HALF : (h + 1) * HALF],
                            start=True,
                            stop=True,
                            tile_position=(0, 32 * s),
                        )
                    nc.vector.tensor_copy(
                        out=w4[:, j, h * (FL // 2) : (h + 1) * (FL // 2), :],
                        in_=pw[:],
                    )

            # ---- Phase 2: stream connectivity, accumulate output -----------
            import contextlib

            loop_ctx = (
                tc.For_i(0, loop_repeat, 1)
                if loop_repeat > 1
                else contextlib.nullcontext()
            )
            with loop_ctx:
                for ab in range(n_ablk):
                    a0 = ab * a_blk
                    ct = cpool.tile([128, a_blk, NL2 * FL], _DT, tag="ct")
                    for s in range(S_PER_CORE):
                        nc.sync.dma_start(
                            out=ct[32 * s : 32 * s + 32],
                            in_=c[s, a0 : a0 + a_blk].rearrange(
                                "a (nh nl) f -> nh a (nl f)", nl=NL2
                            ),
                        )
                    pos = []
                    for s in range(S_PER_CORE):
                        po = psum.tile([OUT_DEPTH, a_blk], _DT, tag="po")
                        pos.append(po)
                        nc.tensor.matmul(
                            po[:],
                            lhsT=fb_sb[:],
                            rhs=bt_sbs[s][:, a0 : a0 + a_blk],
                            start=True,
                            stop=False,
                            tile_position=(0, 0),
                        )
                    for f in range(FL):
                        for j in range(NL2):
                            for s in range(S_PER_CORE):
                                nc.tensor.matmul(
                                    pos[s][:],
                                    lhsT=w4[32 * s : 32 * s + 32, j, f, :],
                                    rhs=ct[32 * s : 32 * s + 32, :, j * FL + f],
                                    start=False,
                                    stop=(f == FL - 1 and j == NL2 - 1),
                                    tile_position=(32 * s, 0),
                                )
                    for s in range(S_PER_CORE):
                        ot = outp.tile([OUT_DEPTH, a_blk], _DT, tag="ot")
                        nc.vector.tensor_copy(out=ot[:], in_=pos[s][:])
                        nc.sync.dma_start(
                            out=out_t[s, :, a0 : a0 + a_blk], in_=ot[:]
                        )

    _hoist_extra_waits(nc)
    return nc


def _build_bass_v3(c_bufs=1, po_bufs=6, loop_repeat=1):
    """v3: 768-byte DMA runs (n = 16*nh + nl) with the partition quarters
    holding the four 128-atom chunks of ONE sample (p = 32*q + nh).  Matmuls
    are K=32 at tile_position row group 32*q with N=128, 4-way concurrent;
    per-sample W is built once at quarter 0 and replicated to the other
    quarters by SBUF->SBUF DMA (cross-partition copies are DMA-only)."""
    nc = bass.Bass()
    c = nc.dram_tensor(
        "c", (S_PER_CORE, N_ATOMS, N_ATOMS, FL), _DT, kind="ExternalInput"
    )
    xt = nc.dram_tensor(
        "xt", (S_PER_CORE, IN_DEPTH, N_ATOMS), _DT, kind="ExternalInput"
    )
    bt = nc.dram_tensor("bt", (S_PER_CORE, 2 * FL, N_ATOMS), _DT, kind="ExternalInput")
    ftf = nc.dram_tensor("ftf", (IN_DEPTH, FL * OUT_DEPTH), _DT, kind="ExternalInput")
    fb = nc.dram_tensor("fb", (2 * FL, OUT_DEPTH), _DT, kind="ExternalInput")
    out_t = nc.dram_tensor(
        "out_t", (S_PER_CORE, OUT_DEPTH, N_ATOMS), _DT, kind="ExternalOutput"
    )

    HALF = FL * OUT_DEPTH // 2  # 384
    AQ = N_ATOMS // 4  # 128 atoms per partition quarter

    with TileContext(nc) as tc:
        with (
            tc.tile_pool(name="consts", bufs=1) as consts,
            tc.tile_pool(name="cpool", bufs=c_bufs) as cpool,
            tc.tile_pool(name="wpool", bufs=1) as wpool,
            tc.tile_pool(name="small", bufs=S_PER_CORE) as small,
            tc.tile_pool(name="outp", bufs=4) as outp,
            tc.tile_pool(name="psum", bufs=po_bufs, space="PSUM") as psum,
            tc.tile_pool(name="psumw", bufs=2, space="PSUM") as psumw,
        ):
            ftf_sb = consts.tile([IN_DEPTH, FL * OUT_DEPTH], _DT)
            nc.sync.dma_start(out=ftf_sb[:], in_=ftf[:])
            fb_sb = consts.tile([2 * FL, OUT_DEPTH], _DT)
            nc.sync.dma_start(out=fb_sb[:], in_=fb[:])

            xt_sbs, bt_sbs = [], []
            for s in range(S_PER_CORE):
                xt_sb = small.tile([IN_DEPTH, N_ATOMS], _DT, tag="xt")
                nc.sync.dma_start(out=xt_sb[:], in_=xt[s])
                xt_sbs.append(xt_sb)
                bt_sb = small.tile([2 * FL, N_ATOMS], _DT, tag="bt")
                nc.sync.dma_start(out=bt_sb[:], in_=bt[s])
                bt_sbs.append(bt_sb)

            import contextlib

            loop_ctx = (
                tc.For_i(0, loop_repeat, 1)
                if loop_repeat > 1
                else contextlib.nullcontext()
            )
            with loop_ctx:
                for s in range(S_PER_CORE):
                    # ---- W_s[n=16*nh+j, f, o] at quarter 0, then replicate --
                    w4 = wpool.tile([128, NL2, FL, OUT_DEPTH], _DT, tag="w4")
                    for j in range(NL2):
                        for h in range(2):
                            pw = psumw.tile([NH2, FL // 2, OUT_DEPTH], _DT, tag="pw")
                            nc.tensor.matmul(
                                pw[:],
                                lhsT=xt_sbs[s][:, j::NL2],  # [64, 32]
                                rhs=ftf_sb[:, h * HALF : (h + 1) * HALF],
                                start=True,
                                stop=True,
                                tile_position=(0, 0),
                            )
                            nc.vector.tensor_copy(
                                out=w4[:NH2, j, h * (FL // 2) : (h + 1) * (FL // 2), :],
                                in_=pw[:],
                            )
                    for q in range(1, 4):
                        nc.sync.dma_start(
                            out=w4[32 * q : 32 * q + 32], in_=w4[0:32]
                        )

                    # ---- C stream: quarter q holds atoms [128q, 128q+128) --
                    ct = cpool.tile([128, AQ, NL2 * FL], _DT, tag="ct")
                    for q in range(4):
                        nc.sync.dma_start(
                            out=ct[32 * q : 32 * q + 32],
                            in_=c[s, AQ * q : AQ * (q + 1)].rearrange(
                                "a (nh nl) f -> nh a (nl f)", nl=NL2
                            ),
                        )
                    pos = []
                    for q in range(4):
                        po = psum.tile([OUT_DEPTH, AQ], _DT, tag="po")
                        pos.append(po)
                        nc.tensor.matmul(
                            po[:],
                            lhsT=fb_sb[:],
                            rhs=bt_sbs[s][:, AQ * q : AQ * (q + 1)],
                            start=True,
                            stop=False,
                            tile_position=(0, 0),
                        )
                    for f in range(FL):
                        for j in range(NL2):
                            for q in range(4):
                                nc.tensor.matmul(
                                    pos[q][:],
                                    lhsT=w4[32 * q : 32 * q + 32, j, f, :],
                                    rhs=ct[32 * q : 32 * q + 32, :, j * FL + f],
                                    start=False,
                                    stop=(f == FL - 1 and j == NL2 - 1),
                                    tile_position=(32 * q, 0),
                                )
                    for q in range(4):
                        ot = outp.tile([OUT_DEPTH, AQ], _DT, tag="ot")
                        nc.vector.tensor_copy(out=ot[:], in_=pos[q][:])
                        nc.sync.dma_start(
                            out=out_t[s, :, AQ * q : AQ * (q + 1)], in_=ot[:]
                        )

    _hoist_extra_waits(nc)
    return nc


def _build_bass_v4(a_blk=128, c_bufs=3, c2_bufs=2, po_bufs=4, loop_repeat=1, dma_split=True):
    """v4: strided DMA load [nh, a, nl, f] (192-B runs), then an on-chip
    free-dim permute to [nh, nl, f, a] split across DVE and ACT (engines read
    strided at full rate; the PE's moving-operand fetch does not - a strided
    rhs streams ~4x slower, measured).  Matmuls then read contiguous [128, a]
    slices, split-K across the two 64-row col groups of the PE (concurrent),
    with a final DVE add of the two halves."""
    nc = bass.Bass()
    c = nc.dram_tensor(
        "c", (S_PER_CORE, N_ATOMS, N_ATOMS, FL), _DT, kind="ExternalInput"
    )
    xt = nc.dram_tensor(
        "xt", (S_PER_CORE, IN_DEPTH, N_ATOMS), _DT, kind="ExternalInput"
    )
    bt = nc.dram_tensor("bt", (S_PER_CORE, 2 * FL, N_ATOMS), _DT, kind="ExternalInput")
    ftf = nc.dram_tensor("ftf", (IN_DEPTH, FL * OUT_DEPTH), _DT, kind="ExternalInput")
    fb = nc.dram_tensor("fb", (2 * FL, OUT_DEPTH), _DT, kind="ExternalInput")
    out_t = nc.dram_tensor(
        "out_t", (S_PER_CORE, OUT_DEPTH, N_ATOMS), _DT, kind="ExternalOutput"
    )

    HALF = FL * OUT_DEPTH // 2  # 384
    n_ablk = N_ATOMS // a_blk
    KCH = FL * NL  # 48 contraction chunks of K=128

    with TileContext(nc) as tc:
        with (
            tc.tile_pool(name="consts", bufs=1) as consts,
            tc.tile_pool(name="cpool", bufs=c_bufs) as cpool,
            tc.tile_pool(name="c2pool", bufs=c2_bufs) as c2pool,
            tc.tile_pool(name="wpool", bufs=S_PER_CORE) as wpool,
            tc.tile_pool(name="small", bufs=S_PER_CORE) as small,
            tc.tile_pool(name="outp", bufs=4) as outp,
            tc.tile_pool(name="psum", bufs=po_bufs, space="PSUM") as psum,
            tc.tile_pool(name="psumw", bufs=2, space="PSUM") as psumw,
        ):
            ftf_sb = consts.tile([IN_DEPTH, FL * OUT_DEPTH], _DT)
            nc.sync.dma_start(out=ftf_sb[:], in_=ftf[:])
            fb_sb = consts.tile([2 * FL, OUT_DEPTH], _DT)
            nc.sync.dma_start(out=fb_sb[:], in_=fb[:])

            xt_sbs, bt_sbs = [], []
            for s in range(S_PER_CORE):
                xt_sb = small.tile([IN_DEPTH, N_ATOMS], _DT, tag="xt")
                nc.sync.dma_start(out=xt_sb[:], in_=xt[s])
                xt_sbs.append(xt_sb)
                bt_sb = small.tile([2 * FL, N_ATOMS], _DT, tag="bt")
                nc.sync.dma_start(out=bt_sb[:], in_=bt[s])
                bt_sbs.append(bt_sb)

            # ---- Phase 1: W[nh, j, f, o] for all samples -------------------
            w_sbs = []
            for s in range(S_PER_CORE):
                w_sb = wpool.tile([NH, NL, FL, OUT_DEPTH], _DT, tag="w")
                w_sbs.append(w_sb)
                for j in range(NL):
                    for h in range(2):
                        pw = psumw.tile([NH, FL // 2, OUT_DEPTH], _DT, tag="pw")
                        nc.tensor.matmul(
                            pw[:],
                            lhsT=xt_sbs[s][:, j::NL],
                            rhs=ftf_sb[:, h * HALF : (h + 1) * HALF],
                            start=True,
                            stop=True,
                        )
                        nc.vector.tensor_copy(
                            out=w_sb[:, j, h * (FL // 2) : (h + 1) * (FL // 2), :],
                            in_=pw[:],
                        )

            # ---- Phase 2 ----------------------------------------------------
            import contextlib

            loop_ctx = (
                tc.For_i(0, loop_repeat, 1)
                if loop_repeat > 1
                else contextlib.nullcontext()
            )
            with loop_ctx:
                for s in range(S_PER_CORE):
                    for ab in range(n_ablk):
                        a0 = ab * a_blk
                        ct = cpool.tile([NH, a_blk, NL, FL], _DT, tag="ct")
                        cin = c[s, a0 : a0 + a_blk].rearrange(
                            "a (nh nl) f -> nh a nl f", nl=NL
                        )
                        if dma_split:
                            hh = a_blk // 2
                            nc.sync.dma_start(out=ct[:, :hh], in_=cin[:, :hh])
                            nc.scalar.dma_start(out=ct[:, hh:], in_=cin[:, hh:])
                        else:
                            nc.sync.dma_start(out=ct[:], in_=cin)
                        # permute (a, nl, f) -> (nl, f, a); DVE 2/3, ACT 1/3
                        ct2 = c2pool.tile([NH, NL, FL, a_blk], _DT, tag="ct2")
                        nc.vector.tensor_copy(
                            out=ct2[:, 0:3],
                            in_=ct[:, :, 0:3].rearrange("p a j f -> p j f a"),
                        )
                        nc.scalar.copy(
                            out=ct2[:, 3:4],
                            in_=ct[:, :, 3:4].rearrange("p a j f -> p j f a"),
                        )

                        po = psum.tile([OUT_DEPTH, a_blk], _DT, tag="po")
                        nc.tensor.matmul(
                            po[:],
                            lhsT=fb_sb[:],
                            rhs=bt_sbs[s][:, a0 : a0 + a_blk],
                            start=True,
                            stop=False,
                        )
                        for k in range(KCH):
                            j, f = k // FL, k % FL
                            nc.tensor.matmul(
                                po[:],
                                lhsT=w_sbs[s][:, j, f, :],
                                rhs=ct2[:, j, f, :],
                                start=False,
                                stop=(k == KCH - 1),
                            )
                        ot = outp.tile([OUT_DEPTH, a_blk], _DT, tag="ot")
                        nc.vector.tensor_copy(out=ot[:], in_=po[:])
                        nc.scalar.dma_start(
                            out=out_t[s, :, a0 : a0 + a_blk], in_=ot[:]
                        )

    _hoist_extra_waits(nc)
    return nc


NKCH = N_ATOMS * FL // 128  # 48 K-chunks of 128 over the (f, n) contraction


def _build_bass_v5(dma_splits=8, c_bufs=6, po_bufs=2, loop_repeat=1, packed=False,
                   skip_mm=False, dma_frac=1, load_engs=("sync",),
                   store_eng="scalar", half_tiles=True):
    # skip_mm / dma_frac: TIMING PROBES ONLY (wrong results): drop the conn
    # matmuls, or DMA only 1/dma_frac of each sample's conn tile.
    """v5: host pre-transposes conn to [s, (f n), a] and casts to bf16, so the
    contraction dim lands directly on partitions with fully contiguous 1 KB
    runs (line-rate DMA, half the bytes of fp32) and no on-chip permute.
    Chunk c of rows [128c, 128c+128) is (f = c//4, n-block = c%4); W tiles are
    built per (sample, n-block) as [128 n, 12 f, 64 o] so lhsT slices match.
    bf16 moving data streams the PE at 1 cycle/row (fp32 is 4)."""
    nc = bass.Bass()
    _BF = mybir.dt.bfloat16
    # packed: host lays conn out as [s, p, c, a] so each partition's whole
    # (c, a) block is one contiguous HBM run (49152 B -> 1 descriptor).
    cb_shape = (
        (S_PER_CORE, 128, NKCH, N_ATOMS) if packed
        else (S_PER_CORE, N_ATOMS * FL, N_ATOMS)
    )
    cb = nc.dram_tensor("cb", cb_shape, _BF, kind="ExternalInput")
    xt = nc.dram_tensor(
        "xt", (S_PER_CORE, IN_DEPTH, N_ATOMS), _DT, kind="ExternalInput"
    )
    btb = nc.dram_tensor(
        "btb", (S_PER_CORE, 2 * FL, N_ATOMS), _BF, kind="ExternalInput"
    )
    ftf = nc.dram_tensor("ftf", (IN_DEPTH, FL * OUT_DEPTH), _DT, kind="ExternalInput")
    fbb = nc.dram_tensor("fbb", (2 * FL, OUT_DEPTH), _BF, kind="ExternalInput")
    out_t = nc.dram_tensor(
        "out_t", (S_PER_CORE, OUT_DEPTH, N_ATOMS), _DT, kind="ExternalOutput"
    )

    HALF = FL * OUT_DEPTH // 2  # 384 = one psum bank at 128 partitions
    NB = N_ATOMS // 128  # 4 n-blocks per sample

    with TileContext(nc) as tc:
        with (
            tc.tile_pool(name="consts", bufs=1) as consts,
            tc.tile_pool(name="cpool", bufs=c_bufs) as cpool,
            tc.tile_pool(name="wpool", bufs=S_PER_CORE * NB) as wpool,
            tc.tile_pool(name="small", bufs=S_PER_CORE) as small,
            tc.tile_pool(name="outp", bufs=3) as outp,
            tc.tile_pool(name="psum", bufs=po_bufs, space="PSUM") as psum,
            tc.tile_pool(name="psumw", bufs=2, space="PSUM") as psumw,
        ):
            ftf_sb = consts.tile([IN_DEPTH, FL * OUT_DEPTH], _DT)
            nc.sync.dma_start(out=ftf_sb[:], in_=ftf[:])
            fbb_sb = consts.tile([2 * FL, OUT_DEPTH], _BF)
            nc.sync.dma_start(out=fbb_sb[:], in_=fbb[:])

            xt_sbs, bt_sbs = [], []
            for s in range(S_PER_CORE):
                xt_sb = small.tile([IN_DEPTH, N_ATOMS], _DT, tag="xt")
                nc.sync.dma_start(out=xt_sb[:], in_=xt[s])
                xt_sbs.append(xt_sb)
                bt_sb = small.tile([2 * FL, N_ATOMS], _BF, tag="bt")
                nc.sync.dma_start(out=bt_sb[:], in_=btb[s])
                bt_sbs.append(bt_sb)

            # ---- Phase 1: W[(s, nb)][n128, f12, o64] bf16 -------------------
            w_sbs = {}
            for s in range(S_PER_CORE):
                for nb in range(NB):
                    w_sb = wpool.tile([128, FL, OUT_DEPTH], _BF, tag="w")
                    w_sbs[(s, nb)] = w_sb
                    for h in range(2):
                        pw = psumw.tile([128, FL // 2, OUT_DEPTH], _DT, tag="pw")
                        nc.tensor.matmul(
                            pw[:],
                            lhsT=xt_sbs[s][:, nb * 128 : (nb + 1) * 128],
                            rhs=ftf_sb[:, h * HALF : (h + 1) * HALF],
                            start=True,
                            stop=True,
                        )
                        nc.vector.tensor_copy(
                            out=w_sb[:, h * (FL // 2) : (h + 1) * (FL // 2), :],
                            in_=pw[:],
                        )

            # ---- Phase 2: stream conn chunks, accumulate output ------------
            import contextlib

            loop_ctx = (
                tc.For_i(0, loop_repeat, 1)
                if loop_repeat > 1
                else contextlib.nullcontext()
            )
            ksplit = NKCH // dma_splits
            eng_map = {"sync": nc.sync, "scalar": nc.scalar, "gpsimd": nc.gpsimd}
            dma_engs = [eng_map[e] for e in load_engs]
            store_e = eng_map[store_eng]
            n_ctile = 2 if half_tiles else 1
            kt = NKCH // n_ctile  # chunks per ct tile
            with loop_ctx:
                for s in range(S_PER_CORE):
                    cts = []
                    for h in range(n_ctile):
                        ct = cpool.tile([128, kt, N_ATOMS], _BF, tag="ct")
                        cts.append(ct)
                        for qq in range(dma_splits // n_ctile):
                            q = h * (dma_splits // n_ctile) + qq
                            ke = ksplit * q + ksplit // dma_frac
                            if packed:
                                src = cb[s, :, ksplit * q : ke]
                            else:
                                r0 = 128 * ksplit * q
                                src = cb[s, r0 : r0 + 128 * (ksplit // dma_frac)].rearrange(
                                    "(c p) a -> p c a", p=128
                                )
                            dma_engs[q % len(dma_engs)].dma_start(
                                out=ct[:, ksplit * qq : ksplit * qq + (ke - ksplit * q)],
                                in_=src,
                            )
                    po = psum.tile([OUT_DEPTH, N_ATOMS], _DT, tag="po")
                    nc.tensor.matmul(
                        po[:],
                        lhsT=fbb_sb[:],
                        rhs=bt_sbs[s][:],
                        start=True,
                        stop=skip_mm,
                    )
                    for k in range(NKCH if not skip_mm else 0):
                        f, nb = k // NB, k % NB
                        nc.tensor.matmul(
                            po[:],
                            lhsT=w_sbs[(s, nb)][:, f, :],
                            rhs=cts[k // kt][:, k % kt, :],
                            start=False,
                            stop=(k == NKCH - 1),
                        )
                    ot = outp.tile([OUT_DEPTH, N_ATOMS], _DT, tag="ot")
                    nc.vector.tensor_copy(out=ot[:], in_=po[:])
                    store_e.dma_start(out=out_t[s], in_=ot[:])

    _hoist_extra_waits(nc)
    return nc


def _hoist_extra_waits(nc):
    """This walrus build rejects any instruction struct carrying more than one
    semaphore wait ("Too many sync wait commands"); Tile freely attaches
    several.  Waits execute in engine-queue order, so hoisting the extras onto
    NoOps inserted directly before the instruction on the same engine is
    semantically identical.  One wait per NoOp."""
    for f in nc.m.functions:
        for blk in f.blocks:
            insts = blk.instructions
            rebuilt = []
            changed = False
            for inst in insts:
                si = inst.sync_info
                if si is not None and len(si.on_wait) > 1:
                    for w in list(si.on_wait)[:-1]:
                        nop = mybir.InstNoOp(
                            name=nc.get_next_instruction_name(),
                            engine=inst.engine,
                            ins=[],
                            outs=[],
                            sync_info=mybir.SyncInfo(on_wait=[w], on_update=[]),
                        )
                        nc.register_instruction(nop)
                        rebuilt.append(nop)
                    inst.sync_info = mybir.SyncInfo(
                        on_wait=[list(si.on_wait)[-1]], on_update=list(si.on_update)
                    )
                    changed = True
                rebuilt.append(inst)
            if changed:
                del insts[:]
                insts.extend(rebuilt)


_CACHED_NC = {}


def _get_nc(repeat=1, version="v1", **kw):
    key = (repeat, version, tuple(sorted(kw.items())))
    if key not in _CACHED_NC:
        if version == "v2":
            _CACHED_NC[key] = _build_bass_v2(**kw)
        elif version == "v3":
            _CACHED_NC[key] = _build_bass_v3(**kw)
        elif version == "v4":
            _CACHED_NC[key] = _build_bass_v4(**kw)
        elif version == "v5":
            _CACHED_NC[key] = _build_bass_v5(**kw)
        elif version == "v6":
            _CACHED_NC[key] = _build_bass_v5(packed=True, **kw)
        else:
            _CACHED_NC[key] = _build_bass(repeat, **kw)
    return _CACHED_NC[key]


def _prep_inputs_v5(node, conn, bond, filters, packed=False):
    """Host-side prep for v5: conn -> [s, (f n), a] bf16 (contraction-major,
    1 KB contiguous runs); bond -> [s, (f j), a] bf16; node/filters fp32."""
    import ml_dtypes

    bf16 = ml_dtypes.bfloat16
    node = np.asarray(node, dtype=np.float32)
    conn = np.asarray(conn, dtype=np.float32)
    bond = np.asarray(bond, dtype=np.float32)
    filters = np.asarray(filters, dtype=np.float32)

    node_t = np.ascontiguousarray(node.transpose(0, 2, 1))
    ftf = np.ascontiguousarray(filters[:, :, :IN_DEPTH].transpose(2, 1, 0)).reshape(
        IN_DEPTH, FL * OUT_DEPTH
    )
    fbb = (
        np.ascontiguousarray(filters[:, :, IN_DEPTH:].transpose(1, 2, 0))
        .reshape(2 * FL, OUT_DEPTH)
        .astype(bf16)
    )
    # cb[s, f*512 + n, a] = conn[s, a, n, f]
    cb = conn.transpose(0, 3, 2, 1).reshape(N_SAMPLES, N_ATOMS * FL, N_ATOMS)
    if packed:
        # [s, (c p), a] -> [s, p, c, a]: one contiguous run per partition
        cb = cb.reshape(N_SAMPLES, NKCH, 128, N_ATOMS).transpose(0, 2, 1, 3)
    btb = bond.transpose(0, 2, 3, 1).reshape(N_SAMPLES, 2 * FL, N_ATOMS)

    in_maps = []
    for k in range(N_CORES):
        s0 = k * S_PER_CORE
        in_maps.append(
            {
                "cb": cb[s0 : s0 + S_PER_CORE].astype(bf16),
                "xt": np.ascontiguousarray(node_t[s0 : s0 + S_PER_CORE]),
                "btb": btb[s0 : s0 + S_PER_CORE].astype(bf16),
                "ftf": ftf,
                "fbb": fbb,
            }
        )
    return in_maps


def _prep_inputs(node, conn, bond, filters):
    """Host-side layout-only prep + per-core sharding."""
    node = np.asarray(node, dtype=np.float32)
    conn = np.asarray(conn, dtype=np.float32)
    bond = np.asarray(bond, dtype=np.float32)
    filters = np.asarray(filters, dtype=np.float32)

    # xt[s, d, n] = node[s, n, d]
    node_t = np.ascontiguousarray(node.transpose(0, 2, 1))
    # bt[s, f*2+j, a] = bond[s, a, f, j]
    bond_t = np.ascontiguousarray(bond.transpose(0, 2, 3, 1)).reshape(
        N_SAMPLES, 2 * FL, N_ATOMS
    )
    # ftf[d, f*64+o] = filters[o, f, d]
    ftf = np.ascontiguousarray(filters[:, :, :IN_DEPTH].transpose(2, 1, 0)).reshape(
        IN_DEPTH, FL * OUT_DEPTH
    )
    # fb[f*2+j, o] = filters[o, f, 64+j]
    fb = np.ascontiguousarray(filters[:, :, IN_DEPTH:].transpose(1, 2, 0)).reshape(
        2 * FL, OUT_DEPTH
    )

    in_maps = []
    for k in range(N_CORES):
        s0 = k * S_PER_CORE
        in_maps.append(
            {
                "c": np.ascontiguousarray(conn[s0 : s0 + S_PER_CORE]),
                "xt": np.ascontiguousarray(node_t[s0 : s0 + S_PER_CORE]),
                "bt": np.ascontiguousarray(bond_t[s0 : s0 + S_PER_CORE]),
                "ftf": ftf,
                "fb": fb,
            }
        )
    return in_maps


# Best-measured variant used by kernel(); see bench logs in the session.
_BEST_VERSION = "v6"


def run(node_property_tensor, connectivity_tensor, bond_property_tensor, filters,
        trace=False, version=None):
    """Run on 8 cores; returns (output [32,512,64], BassKernelResults)."""
    version = version or _BEST_VERSION
    nc = _get_nc(version=version)
    if version == "v5":
        prep = _prep_inputs_v5
    elif version == "v6":
        from functools import partial
        prep = partial(_prep_inputs_v5, packed=True)
    else:
        prep = _prep_inputs
    in_maps = prep(
        node_property_tensor, connectivity_tensor, bond_property_tensor, filters
    )
    res = bass_utils.run_bass_kernel_spmd(
        nc, in_maps, core_ids=list(range(N_CORES)), trace=trace
    )
    out = np.empty((N_SAMPLES, N_ATOMS, OUT_DEPTH), dtype=np.float32)
    for k in range(N_CORES):
        s0 = k * S_PER_CORE
        out[s0 : s0 + S_PER_CORE] = res.results[k]["out_t"].transpose(0, 2, 1)
    return out, res


def kernel(node_property_tensor, connectivity_tensor, bond_property_tensor, filters):
    out, _ = run(
        node_property_tensor, connectivity_tensor, bond_property_tensor, filters
    )
    return out



# revision 19
# speedup vs baseline: 26.7971x; 1.0097x over previous
"""ChemConv (GNN message passing) kernel for Trainium2, 8 NeuronCores.

Reference math (per sample s):
    node_conn[a,f,d] = sum_n conn[a,n,f] * node[n,d]
    out[a,o]         = sum_{f,d} cat(node_conn, bond)[a,f,d] * filters[o,f,d]

Folded form used on device (filters folded into node features):
    W[n,f,o]  = sum_d node[n,d] * filters[o,f,d]          (tiny matmuls)
    out[a,o]  = sum_{n,f} conn[a,(n,f)] * W[(n,f),o]
              + sum_{f,j} bond[a,f,j] * filters[o,f,64+j]

Sharding: data-parallel over the 32 samples -> 4 samples per core x 8 cores.

Best variant (v6): the kernel is HBM-bound on streaming conn (the only big
tensor), so the host pre-arranges conn into exactly the tiles the PE wants:
cb[s, p, c, a] bf16, where chunk c of contraction rows [128c, 128c+128) over
K = (f, n) maps to (f = c//4, n-block = c%4).  Each partition's (c, a) block
is one contiguous HBM run, so conn streams at line rate (~318 GB/s measured
under 8-core load) with no on-chip permute, and the bf16 cast halves HBM
bytes (rel err ~2e-3, gate 2e-2).  Matmuls are then plain K=128 accumulating
GEMMs with 512-wide bf16 moving data (1 cycle/row; fp32 is 4).  W tiles
[128 (n), 12 (f), 64 (o)] bf16 are built on chip once per call (phase 1) to
match the chunk order.  Conn loads ride one HWDGE ring (sync) in half-sample
tiles x6 bufs so the load stream never stalls; output stores ride the other
ring (scalar).  Older variants (v1-v5) kept for reference / probes.

Walrus quirk: a Matmult instruction (its LDWEIGHTS struct) can carry at most
ONE semaphore wait; Tile freely attaches several.  After Tile scheduling we
hoist the extra waits onto NoOps inserted directly before the matmul on the
same engine queue - semantically identical (waits execute in queue order).
"""

import sys

import numpy as np

try:
    import concourse.bass as bass
except ImportError:  # pragma: no cover
    sys.path.append("/opt/trn_rl_repo")
    import concourse.bass as bass

import concourse.mybir as mybir
from concourse import bass_utils
from concourse.tile import TileContext

N_SAMPLES, N_ATOMS = 32, 512
IN_DEPTH, OUT_DEPTH, FL = 64, 64, 12
N_CORES = 8
S_PER_CORE = N_SAMPLES // N_CORES  # 4

NL = 4  # low bits of n folded into the free dim (192-byte DMA runs)
NH = N_ATOMS // NL  # 128 partitions
A_BLK = 256  # atoms per connectivity tile
N_ABLK = N_ATOMS // A_BLK

PSW_BUFS = 4  # psum banks for W building
PO_BUFS = 4  # psum banks for output accumulation

_DT = mybir.dt.float32


def _build_bass(repeat=1, a_blk=A_BLK, c_bufs=2, dma_engines=("sync",),
                merge_ap=False, dma_parts=1, loop_repeat=1):
    """repeat > 1 re-runs phase 2 (the C stream + matmuls) that many times
    inside the NEFF - output is identical; used only to amortize the host
    dispatch overhead when measuring device-side time.

    a_blk: atoms per connectivity tile; c_bufs: tile-pool bufs for them.
    dma_engines: rotation of descriptor-generation engines for the C stream
    ("sync" and "scalar" are the two independent HWDGE rings, "gpsimd" is the
    SWDGE Q7 path).  dma_parts: split each tile's DMA into this many
    dma_starts over disjoint atom ranges, rotating engines.
    merge_ap: use the 3D [nh, a, (nl f)] access pattern instead of 4D."""
    nc = bass.Bass()
    c = nc.dram_tensor(
        "c", (S_PER_CORE, N_ATOMS, N_ATOMS, FL), _DT, kind="ExternalInput"
    )
    xt = nc.dram_tensor(
        "xt", (S_PER_CORE, IN_DEPTH, N_ATOMS), _DT, kind="ExternalInput"
    )
    bt = nc.dram_tensor("bt", (S_PER_CORE, 2 * FL, N_ATOMS), _DT, kind="ExternalInput")
    ftf = nc.dram_tensor("ftf", (IN_DEPTH, FL * OUT_DEPTH), _DT, kind="ExternalInput")
    fb = nc.dram_tensor("fb", (2 * FL, OUT_DEPTH), _DT, kind="ExternalInput")
    out_t = nc.dram_tensor(
        "out_t", (S_PER_CORE, OUT_DEPTH, N_ATOMS), _DT, kind="ExternalOutput"
    )

    HALF = FL * OUT_DEPTH // 2  # 384 columns per W-build matmul (one psum bank)

    with TileContext(nc) as tc:
        with (
            tc.tile_pool(name="consts", bufs=1) as consts,
            tc.tile_pool(name="cpool", bufs=c_bufs) as cpool,
            tc.tile_pool(name="wpool", bufs=S_PER_CORE) as wpool,
            tc.tile_pool(name="small", bufs=S_PER_CORE) as small,
            tc.tile_pool(name="outp", bufs=3) as outp,
            tc.tile_pool(name="psum", bufs=PO_BUFS, space="PSUM") as psum,
            tc.tile_pool(name="psumw", bufs=PSW_BUFS, space="PSUM") as psumw,
        ):
            ftf_sb = consts.tile([IN_DEPTH, FL * OUT_DEPTH], _DT)
            ftf_dma = nc.sync.dma_start(out=ftf_sb[:], in_=ftf[:])
            fb_sb = consts.tile([2 * FL, OUT_DEPTH], _DT)
            fb_dma = nc.sync.dma_start(out=fb_sb[:], in_=fb[:])

            xt_sbs, bt_sbs, xt_dmas, bt_dmas = [], [], [], []
            for s in range(S_PER_CORE):
                xt_sb = small.tile([IN_DEPTH, N_ATOMS], _DT, tag="xt")
                xt_dmas.append(nc.sync.dma_start(out=xt_sb[:], in_=xt[s]))
                xt_sbs.append(xt_sb)
                bt_sb = small.tile([2 * FL, N_ATOMS], _DT, tag="bt")
                bt_dmas.append(nc.sync.dma_start(out=bt_sb[:], in_=bt[s]))
                bt_sbs.append(bt_sb)

            # ---- Phase 1: W for all samples --------------------------------
            # W[nh, nl, f, o] = sum_d node[4*nh + nl, d] * filters[o, f, d]
            w_sbs = []
            for s in range(S_PER_CORE):
                w_sb = wpool.tile([NH, NL, FL, OUT_DEPTH], _DT, tag="w")
                w_sbs.append(w_sb)
                for j in range(NL):
                    for h in range(2):
                        pw = psumw.tile([NH, FL // 2, OUT_DEPTH], _DT, tag="pw")
                        nc.tensor.matmul(
                            pw[:],
                            lhsT=xt_sbs[s][:, j::NL],  # cols are n = 4*nh+j
                            rhs=ftf_sb[:, h * HALF : (h + 1) * HALF],
                            start=True,
                            stop=True,
                        )
                        nc.vector.tensor_copy(
                            out=w_sb[:, j, h * (FL // 2) : (h + 1) * (FL // 2), :],
                            in_=pw[:],
                        )

            # ---- Phase 2: stream connectivity, accumulate output -----------
            n_ablk = N_ATOMS // a_blk
            eng_map = {
                "sync": nc.sync,
                "scalar": nc.scalar,
                "gpsimd": nc.gpsimd,
            }
            dma_rot = 0

            import contextlib

            loop_ctx = (
                tc.For_i(0, loop_repeat, 1)
                if loop_repeat > 1
                else contextlib.nullcontext()
            )
            with loop_ctx:
                phase2(
                    nc, tc, repeat, a_blk, n_ablk, eng_map, dma_rot, merge_ap,
                    dma_parts, dma_engines, cpool, psum, outp, c, out_t,
                    fb_sb, bt_sbs, w_sbs,
                )

    _hoist_extra_waits(nc)
    return nc


def phase2(nc, tc, repeat, a_blk, n_ablk, eng_map, dma_rot, merge_ap,
           dma_parts, dma_engines, cpool, psum, outp, c, out_t,
           fb_sb, bt_sbs, w_sbs):
    if True:
        if True:
            for s in [s for _ in range(repeat) for s in range(S_PER_CORE)]:
                for ab in range(n_ablk):
                    a0 = ab * a_blk
                    if merge_ap:
                        ct = cpool.tile([NH, a_blk, NL * FL], _DT, tag="ct")
                        in_full = c[s, a0 : a0 + a_blk].rearrange(
                            "a (nh nl) f -> nh a (nl f)", nl=NL
                        )
                    else:
                        ct = cpool.tile([NH, a_blk, NL, FL], _DT, tag="ct")
                        in_full = c[s, a0 : a0 + a_blk].rearrange(
                            "a (nh nl) f -> nh a nl f", nl=NL
                        )
                    part = a_blk // dma_parts
                    for pi in range(dma_parts):
                        eng = eng_map[dma_engines[dma_rot % len(dma_engines)]]
                        dma_rot += 1
                        eng.dma_start(
                            out=ct[:, pi * part : (pi + 1) * part],
                            in_=in_full[:, pi * part : (pi + 1) * part],
                        )

                    po = psum.tile([OUT_DEPTH, a_blk], _DT, tag="po")
                    # bond contribution first: out[o,a] += fb[fj,o]^T @ bt[fj,a]
                    nc.tensor.matmul(
                        po[:],
                        lhsT=fb_sb[:],
                        rhs=bt_sbs[s][:, a0 : a0 + a_blk],
                        start=True,
                        stop=False,
                    )
                    for f in range(FL):
                        for j in range(NL):
                            rhs = (
                                ct[:, :, j * FL + f]
                                if merge_ap
                                else ct[:, :, j, f]
                            )
                            nc.tensor.matmul(
                                po[:],
                                lhsT=w_sbs[s][:, j, f, :],  # [128, 64]
                                rhs=rhs,  # [128, a_blk]
                                start=False,
                                stop=(f == FL - 1 and j == NL - 1),
                            )
                    ot = outp.tile([OUT_DEPTH, a_blk], _DT, tag="ot")
                    nc.vector.tensor_copy(out=ot[:], in_=po[:])
                    nc.sync.dma_start(
                        out=out_t[s, :, a0 : a0 + a_blk], in_=ot[:]
                    )


NL2 = 16  # v2: low bits of n in the free dim -> 768-byte runs (no RMW penalty)
NH2 = N_ATOMS // NL2  # 32 partitions per sample


def _build_bass_v2(a_blk=64, c_bufs=2, po_bufs=6, loop_repeat=1):
    """v2: 768-byte DMA runs via n = 16*nh + nl, with the 4 samples packed
    across the 4 partition quarters (p = 32*s + nh).  The four 32-partition
    DMAs per atom chunk cover complementary engine sets of the SBUF port
    swizzle, and the K=32 matmuls use tile_position row groups (4 concurrent)
    with each sample's W stored in its partition quarter of one W tile."""
    nc = bass.Bass()
    c = nc.dram_tensor(
        "c", (S_PER_CORE, N_ATOMS, N_ATOMS, FL), _DT, kind="ExternalInput"
    )
    xt = nc.dram_tensor(
        "xt", (S_PER_CORE, IN_DEPTH, N_ATOMS), _DT, kind="ExternalInput"
    )
    bt = nc.dram_tensor("bt", (S_PER_CORE, 2 * FL, N_ATOMS), _DT, kind="ExternalInput")
    ftf = nc.dram_tensor("ftf", (IN_DEPTH, FL * OUT_DEPTH), _DT, kind="ExternalInput")
    fb = nc.dram_tensor("fb", (2 * FL, OUT_DEPTH), _DT, kind="ExternalInput")
    out_t = nc.dram_tensor(
        "out_t", (S_PER_CORE, OUT_DEPTH, N_ATOMS), _DT, kind="ExternalOutput"
    )

    HALF = FL * OUT_DEPTH // 2  # 384
    n_ablk = N_ATOMS // a_blk

    with TileContext(nc) as tc:
        with (
            tc.tile_pool(name="consts", bufs=1) as consts,
            tc.tile_pool(name="cpool", bufs=c_bufs) as cpool,
            tc.tile_pool(name="wpool", bufs=1) as wpool,
            tc.tile_pool(name="small", bufs=S_PER_CORE) as small,
            tc.tile_pool(name="outp", bufs=4) as outp,
            tc.tile_pool(name="psum", bufs=po_bufs, space="PSUM") as psum,
            tc.tile_pool(name="psumw", bufs=2, space="PSUM") as psumw,
        ):
            ftf_sb = consts.tile([IN_DEPTH, FL * OUT_DEPTH], _DT)
            nc.sync.dma_start(out=ftf_sb[:], in_=ftf[:])
            fb_sb = consts.tile([2 * FL, OUT_DEPTH], _DT)
            nc.sync.dma_start(out=fb_sb[:], in_=fb[:])

            xt_sbs, bt_sbs = [], []
            for s in range(S_PER_CORE):
                xt_sb = small.tile([IN_DEPTH, N_ATOMS], _DT, tag="xt")
                nc.sync.dma_start(out=xt_sb[:], in_=xt[s])
                xt_sbs.append(xt_sb)
                bt_sb = small.tile([2 * FL, N_ATOMS], _DT, tag="bt")
                nc.sync.dma_start(out=bt_sb[:], in_=bt[s])
                bt_sbs.append(bt_sb)

            # ---- Phase 1: W4[p=32s+nh, j, f, o] = W_s[n=16*nh+j, f, o] ------
            w4 = wpool.tile([128, NL2, FL, OUT_DEPTH], _DT)
            for j in range(NL2):
                for h in range(2):
                    pw = psumw.tile([128, FL // 2, OUT_DEPTH], _DT, tag="pw")
                    for s in range(S_PER_CORE):
                        nc.tensor.matmul(
                            pw[32 * s : 32 * s + 32],
                            lhsT=xt_sbs[s][:, j::NL2],  # [64, 32] cols n=16nh+j
                            rhs=ftf_sb[:, h * HALF : (h + 1) * HALF],
                            start=True,
                            stop=True,
                            tile_position=(0, 32 * s),
                        )
                    nc.vector.tensor_copy(
                        out=w4[:, j, h * (FL // 2) : (h + 1) * (FL // 2), :],
                        in_=pw[:],
                    )

            # ---- Phase 2: stream connectivity, accumulate output -----------
            import contextlib

            loop_ctx = (
                tc.For_i(0, loop_repeat, 1)
                if loop_repeat > 1
                else contextlib.nullcontext()
            )
            with loop_ctx:
                for ab in range(n_ablk):
                    a0 = ab * a_blk
                    ct = cpool.tile([128, a_blk, NL2 * FL], _DT, tag="ct")
                    for s in range(S_PER_CORE):
                        nc.sync.dma_start(
                            out=ct[32 * s : 32 * s + 32],
                            in_=c[s, a0 : a0 + a_blk].rearrange(
                                "a (nh nl) f -> nh a (nl f)", nl=NL2
                            ),
                        )
                    pos = []
                    for s in range(S_PER_CORE):
                        po = psum.tile([OUT_DEPTH, a_blk], _DT, tag="po")
                        pos.append(po)
                        nc.tensor.matmul(
                            po[:],
                            lhsT=fb_sb[:],
                            rhs=bt_sbs[s][:, a0 : a0 + a_blk],
                            start=True,
                            stop=False,
                            tile_position=(0, 0),
                        )
                    for f in range(FL):
                        for j in range(NL2):
                            for s in range(S_PER_CORE):
                                nc.tensor.matmul(
                                    pos[s][:],
                                    lhsT=w4[32 * s : 32 * s + 32, j, f, :],
                                    rhs=ct[32 * s : 32 * s + 32, :, j * FL + f],
                                    start=False,
                                    stop=(f == FL - 1 and j == NL2 - 1),
                                    tile_position=(32 * s, 0),
                                )
                    for s in range(S_PER_CORE):
                        ot = outp.tile([OUT_DEPTH, a_blk], _DT, tag="ot")
                        nc.vector.tensor_copy(out=ot[:], in_=pos[s][:])
                        nc.sync.dma_start(
                            out=out_t[s, :, a0 : a0 + a_blk], in_=ot[:]
                        )

    _hoist_extra_waits(nc)
    return nc


def _build_bass_v3(c_bufs=1, po_bufs=6, loop_repeat=1):
    """v3: 768-byte DMA runs (n = 16*nh + nl) with the partition quarters
    holding the four 128-atom chunks of ONE sample (p = 32*q + nh).  Matmuls
    are K=32 at tile_position row group 32*q with N=128, 4-way concurrent;
    per-sample W is built once at quarter 0 and replicated to the other
    quarters by SBUF->SBUF DMA (cross-partition copies are DMA-only)."""
    nc = bass.Bass()
    c = nc.dram_tensor(
        "c", (S_PER_CORE, N_ATOMS, N_ATOMS, FL), _DT, kind="ExternalInput"
    )
    xt = nc.dram_tensor(
        "xt", (S_PER_CORE, IN_DEPTH, N_ATOMS), _DT, kind="ExternalInput"
    )
    bt = nc.dram_tensor("bt", (S_PER_CORE, 2 * FL, N_ATOMS), _DT, kind="ExternalInput")
    ftf = nc.dram_tensor("ftf", (IN_DEPTH, FL * OUT_DEPTH), _DT, kind="ExternalInput")
    fb = nc.dram_tensor("fb", (2 * FL, OUT_DEPTH), _DT, kind="ExternalInput")
    out_t = nc.dram_tensor(
        "out_t", (S_PER_CORE, OUT_DEPTH, N_ATOMS), _DT, kind="ExternalOutput"
    )

    HALF = FL * OUT_DEPTH // 2  # 384
    AQ = N_ATOMS // 4  # 128 atoms per partition quarter

    with TileContext(nc) as tc:
        with (
            tc.tile_pool(name="consts", bufs=1) as consts,
            tc.tile_pool(name="cpool", bufs=c_bufs) as cpool,
            tc.tile_pool(name="wpool", bufs=1) as wpool,
            tc.tile_pool(name="small", bufs=S_PER_CORE) as small,
            tc.tile_pool(name="outp", bufs=4) as outp,
            tc.tile_pool(name="psum", bufs=po_bufs, space="PSUM") as psum,
            tc.tile_pool(name="psumw", bufs=2, space="PSUM") as psumw,
        ):
            ftf_sb = consts.tile([IN_DEPTH, FL * OUT_DEPTH], _DT)
            nc.sync.dma_start(out=ftf_sb[:], in_=ftf[:])
            fb_sb = consts.tile([2 * FL, OUT_DEPTH], _DT)
            nc.sync.dma_start(out=fb_sb[:], in_=fb[:])

            xt_sbs, bt_sbs = [], []
            for s in range(S_PER_CORE):
                xt_sb = small.tile([IN_DEPTH, N_ATOMS], _DT, tag="xt")
                nc.sync.dma_start(out=xt_sb[:], in_=xt[s])
                xt_sbs.append(xt_sb)
                bt_sb = small.tile([2 * FL, N_ATOMS], _DT, tag="bt")
                nc.sync.dma_start(out=bt_sb[:], in_=bt[s])
                bt_sbs.append(bt_sb)

            import contextlib

            loop_ctx = (
                tc.For_i(0, loop_repeat, 1)
                if loop_repeat > 1
                else contextlib.nullcontext()
            )
            with loop_ctx:
                for s in range(S_PER_CORE):
                    # ---- W_s[n=16*nh+j, f, o] at quarter 0, then replicate --
                    w4 = wpool.tile([128, NL2, FL, OUT_DEPTH], _DT, tag="w4")
                    for j in range(NL2):
                        for h in range(2):
                            pw = psumw.tile([NH2, FL // 2, OUT_DEPTH], _DT, tag="pw")
                            nc.tensor.matmul(
                                pw[:],
                                lhsT=xt_sbs[s][:, j::NL2],  # [64, 32]
                                rhs=ftf_sb[:, h * HALF : (h + 1) * HALF],
                                start=True,
                                stop=True,
                                tile_position=(0, 0),
                            )
                            nc.vector.tensor_copy(
                                out=w4[:NH2, j, h * (FL // 2) : (h + 1) * (FL // 2), :],
                                in_=pw[:],
                            )
                    for q in range(1, 4):
                        nc.sync.dma_start(
                            out=w4[32 * q : 32 * q + 32], in_=w4[0:32]
                        )

                    # ---- C stream: quarter q holds atoms [128q, 128q+128) --
                    ct = cpool.tile([128, AQ, NL2 * FL], _DT, tag="ct")
                    for q in range(4):
                        nc.sync.dma_start(
                            out=ct[32 * q : 32 * q + 32],
                            in_=c[s, AQ * q : AQ * (q + 1)].rearrange(
                                "a (nh nl) f -> nh a (nl f)", nl=NL2
                            ),
                        )
                    pos = []
                    for q in range(4):
                        po = psum.tile([OUT_DEPTH, AQ], _DT, tag="po")
                        pos.append(po)
                        nc.tensor.matmul(
                            po[:],
                            lhsT=fb_sb[:],
                            rhs=bt_sbs[s][:, AQ * q : AQ * (q + 1)],
                            start=True,
                            stop=False,
                            tile_position=(0, 0),
                        )
                    for f in range(FL):
                        for j in range(NL2):
                            for q in range(4):
                                nc.tensor.matmul(
                                    pos[q][:],
                                    lhsT=w4[32 * q : 32 * q + 32, j, f, :],
                                    rhs=ct[32 * q : 32 * q + 32, :, j * FL + f],
                                    start=False,
                                    stop=(f == FL - 1 and j == NL2 - 1),
                                    tile_position=(32 * q, 0),
                                )
                    for q in range(4):
                        ot = outp.tile([OUT_DEPTH, AQ], _DT, tag="ot")
                        nc.vector.tensor_copy(out=ot[:], in_=pos[q][:])
                        nc.sync.dma_start(
                            out=out_t[s, :, AQ * q : AQ * (q + 1)], in_=ot[:]
                        )

    _hoist_extra_waits(nc)
    return nc


def _build_bass_v4(a_blk=128, c_bufs=3, c2_bufs=2, po_bufs=4, loop_repeat=1, dma_split=True):
    """v4: strided DMA load [nh, a, nl, f] (192-B runs), then an on-chip
    free-dim permute to [nh, nl, f, a] split across DVE and ACT (engines read
    strided at full rate; the PE's moving-operand fetch does not - a strided
    rhs streams ~4x slower, measured).  Matmuls then read contiguous [128, a]
    slices, split-K across the two 64-row col groups of the PE (concurrent),
    with a final DVE add of the two halves."""
    nc = bass.Bass()
    c = nc.dram_tensor(
        "c", (S_PER_CORE, N_ATOMS, N_ATOMS, FL), _DT, kind="ExternalInput"
    )
    xt = nc.dram_tensor(
        "xt", (S_PER_CORE, IN_DEPTH, N_ATOMS), _DT, kind="ExternalInput"
    )
    bt = nc.dram_tensor("bt", (S_PER_CORE, 2 * FL, N_ATOMS), _DT, kind="ExternalInput")
    ftf = nc.dram_tensor("ftf", (IN_DEPTH, FL * OUT_DEPTH), _DT, kind="ExternalInput")
    fb = nc.dram_tensor("fb", (2 * FL, OUT_DEPTH), _DT, kind="ExternalInput")
    out_t = nc.dram_tensor(
        "out_t", (S_PER_CORE, OUT_DEPTH, N_ATOMS), _DT, kind="ExternalOutput"
    )

    HALF = FL * OUT_DEPTH // 2  # 384
    n_ablk = N_ATOMS // a_blk
    KCH = FL * NL  # 48 contraction chunks of K=128

    with TileContext(nc) as tc:
        with (
            tc.tile_pool(name="consts", bufs=1) as consts,
            tc.tile_pool(name="cpool", bufs=c_bufs) as cpool,
            tc.tile_pool(name="c2pool", bufs=c2_bufs) as c2pool,
            tc.tile_pool(name="wpool", bufs=S_PER_CORE) as wpool,
            tc.tile_pool(name="small", bufs=S_PER_CORE) as small,
            tc.tile_pool(name="outp", bufs=4) as outp,
            tc.tile_pool(name="psum", bufs=po_bufs, space="PSUM") as psum,
            tc.tile_pool(name="psumw", bufs=2, space="PSUM") as psumw,
        ):
            ftf_sb = consts.tile([IN_DEPTH, FL * OUT_DEPTH], _DT)
            nc.sync.dma_start(out=ftf_sb[:], in_=ftf[:])
            fb_sb = consts.tile([2 * FL, OUT_DEPTH], _DT)
            nc.sync.dma_start(out=fb_sb[:], in_=fb[:])

            xt_sbs, bt_sbs = [], []
            for s in range(S_PER_CORE):
                xt_sb = small.tile([IN_DEPTH, N_ATOMS], _DT, tag="xt")
                nc.sync.dma_start(out=xt_sb[:], in_=xt[s])
                xt_sbs.append(xt_sb)
                bt_sb = small.tile([2 * FL, N_ATOMS], _DT, tag="bt")
                nc.sync.dma_start(out=bt_sb[:], in_=bt[s])
                bt_sbs.append(bt_sb)

            # ---- Phase 1: W[nh, j, f, o] for all samples -------------------
            w_sbs = []
            for s in range(S_PER_CORE):
                w_sb = wpool.tile([NH, NL, FL, OUT_DEPTH], _DT, tag="w")
                w_sbs.append(w_sb)
                for j in range(NL):
                    for h in range(2):
                        pw = psumw.tile([NH, FL // 2, OUT_DEPTH], _DT, tag="pw")
                        nc.tensor.matmul(
                            pw[:],
                            lhsT=xt_sbs[s][:, j::NL],
                            rhs=ftf_sb[:, h * HALF : (h + 1) * HALF],
                            start=True,
                            stop=True,
                        )
                        nc.vector.tensor_copy(
                            out=w_sb[:, j, h * (FL // 2) : (h + 1) * (FL // 2), :],
                            in_=pw[:],
                        )

            # ---- Phase 2 ----------------------------------------------------
            import contextlib

            loop_ctx = (
                tc.For_i(0, loop_repeat, 1)
                if loop_repeat > 1
                else contextlib.nullcontext()
            )
            with loop_ctx:
                for s in range(S_PER_CORE):
                    for ab in range(n_ablk):
                        a0 = ab * a_blk
                        ct = cpool.tile([NH, a_blk, NL, FL], _DT, tag="ct")
                        cin = c[s, a0 : a0 + a_blk].rearrange(
                            "a (nh nl) f -> nh a nl f", nl=NL
                        )
                        if dma_split:
                            hh = a_blk // 2
                            nc.sync.dma_start(out=ct[:, :hh], in_=cin[:, :hh])
                            nc.scalar.dma_start(out=ct[:, hh:], in_=cin[:, hh:])
                        else:
                            nc.sync.dma_start(out=ct[:], in_=cin)
                        # permute (a, nl, f) -> (nl, f, a); DVE 2/3, ACT 1/3
                        ct2 = c2pool.tile([NH, NL, FL, a_blk], _DT, tag="ct2")
                        nc.vector.tensor_copy(
                            out=ct2[:, 0:3],
                            in_=ct[:, :, 0:3].rearrange("p a j f -> p j f a"),
                        )
                        nc.scalar.copy(
                            out=ct2[:, 3:4],
                            in_=ct[:, :, 3:4].rearrange("p a j f -> p j f a"),
                        )

                        po = psum.tile([OUT_DEPTH, a_blk], _DT, tag="po")
                        nc.tensor.matmul(
                            po[:],
                            lhsT=fb_sb[:],
                            rhs=bt_sbs[s][:, a0 : a0 + a_blk],
                            start=True,
                            stop=False,
                        )
                        for k in range(KCH):
                            j, f = k // FL, k % FL
                            nc.tensor.matmul(
                                po[:],
                                lhsT=w_sbs[s][:, j, f, :],
                                rhs=ct2[:, j, f, :],
                                start=False,
                                stop=(k == KCH - 1),
                            )
                        ot = outp.tile([OUT_DEPTH, a_blk], _DT, tag="ot")
                        nc.vector.tensor_copy(out=ot[:], in_=po[:])
                        nc.scalar.dma_start(
                            out=out_t[s, :, a0 : a0 + a_blk], in_=ot[:]
                        )

    _hoist_extra_waits(nc)
    return nc


NKCH = N_ATOMS * FL // 128  # 48 K-chunks of 128 over the (f, n) contraction


def _build_bass_v5(dma_splits=8, c_bufs=6, po_bufs=2, loop_repeat=1, packed=False,
                   skip_mm=False, dma_frac=1, load_engs=("sync",),
                   store_eng="scalar", half_tiles=True, bf16_out=False):
    # skip_mm / dma_frac: TIMING PROBES ONLY (wrong results): drop the conn
    # matmuls, or DMA only 1/dma_frac of each sample's conn tile.
    """v5: host pre-transposes conn to [s, (f n), a] and casts to bf16, so the
    contraction dim lands directly on partitions with fully contiguous 1 KB
    runs (line-rate DMA, half the bytes of fp32) and no on-chip permute.
    Chunk c of rows [128c, 128c+128) is (f = c//4, n-block = c%4); W tiles are
    built per (sample, n-block) as [128 n, 12 f, 64 o] so lhsT slices match.
    bf16 moving data streams the PE at 1 cycle/row (fp32 is 4)."""
    nc = bass.Bass()
    _BF = mybir.dt.bfloat16
    # packed: host lays conn out as [s, p, c, a] so each partition's whole
    # (c, a) block is one contiguous HBM run (49152 B -> 1 descriptor).
    cb_shape = (
        (S_PER_CORE, 128, NKCH, N_ATOMS) if packed
        else (S_PER_CORE, N_ATOMS * FL, N_ATOMS)
    )
    cb = nc.dram_tensor("cb", cb_shape, _BF, kind="ExternalInput")
    xt = nc.dram_tensor(
        "xt", (S_PER_CORE, IN_DEPTH, N_ATOMS), _DT, kind="ExternalInput"
    )
    btb = nc.dram_tensor(
        "btb", (S_PER_CORE, 2 * FL, N_ATOMS), _BF, kind="ExternalInput"
    )
    ftf = nc.dram_tensor("ftf", (IN_DEPTH, FL * OUT_DEPTH), _DT, kind="ExternalInput")
    fbb = nc.dram_tensor("fbb", (2 * FL, OUT_DEPTH), _BF, kind="ExternalInput")
    _OT = _BF if bf16_out else _DT
    out_t = nc.dram_tensor(
        "out_t", (S_PER_CORE, OUT_DEPTH, N_ATOMS), _OT, kind="ExternalOutput"
    )

    HALF = FL * OUT_DEPTH // 2  # 384 = one psum bank at 128 partitions
    NB = N_ATOMS // 128  # 4 n-blocks per sample

    with TileContext(nc) as tc:
        with (
            tc.tile_pool(name="consts", bufs=1) as consts,
            tc.tile_pool(name="cpool", bufs=c_bufs) as cpool,
            tc.tile_pool(name="wpool", bufs=S_PER_CORE * NB) as wpool,
            tc.tile_pool(name="small", bufs=S_PER_CORE) as small,
            tc.tile_pool(name="outp", bufs=3) as outp,
            tc.tile_pool(name="psum", bufs=po_bufs, space="PSUM") as psum,
            tc.tile_pool(name="psumw", bufs=2, space="PSUM") as psumw,
        ):
            ftf_sb = consts.tile([IN_DEPTH, FL * OUT_DEPTH], _DT)
            nc.sync.dma_start(out=ftf_sb[:], in_=ftf[:])
            fbb_sb = consts.tile([2 * FL, OUT_DEPTH], _BF)
            nc.sync.dma_start(out=fbb_sb[:], in_=fbb[:])

            xt_sbs, bt_sbs = [], []
            for s in range(S_PER_CORE):
                xt_sb = small.tile([IN_DEPTH, N_ATOMS], _DT, tag="xt")
                nc.sync.dma_start(out=xt_sb[:], in_=xt[s])
                xt_sbs.append(xt_sb)
                bt_sb = small.tile([2 * FL, N_ATOMS], _BF, tag="bt")
                nc.sync.dma_start(out=bt_sb[:], in_=btb[s])
                bt_sbs.append(bt_sb)

            # ---- Phase 1: W[(s, nb)][n128, f12, o64] bf16 -------------------
            w_sbs = {}
            for s in range(S_PER_CORE):
                for nb in range(NB):
                    w_sb = wpool.tile([128, FL, OUT_DEPTH], _BF, tag="w")
                    w_sbs[(s, nb)] = w_sb
                    for h in range(2):
                        pw = psumw.tile([128, FL // 2, OUT_DEPTH], _DT, tag="pw")
                        nc.tensor.matmul(
                            pw[:],
                            lhsT=xt_sbs[s][:, nb * 128 : (nb + 1) * 128],
                            rhs=ftf_sb[:, h * HALF : (h + 1) * HALF],
                            start=True,
                            stop=True,
                        )
                        nc.vector.tensor_copy(
                            out=w_sb[:, h * (FL // 2) : (h + 1) * (FL // 2), :],
                            in_=pw[:],
                        )

            # ---- Phase 2: stream conn chunks, accumulate output ------------
            import contextlib

            loop_ctx = (
                tc.For_i(0, loop_repeat, 1)
                if loop_repeat > 1
                else contextlib.nullcontext()
            )
            ksplit = NKCH // dma_splits
            eng_map = {"sync": nc.sync, "scalar": nc.scalar, "gpsimd": nc.gpsimd}
            dma_engs = [eng_map[e] for e in load_engs]
            store_e = eng_map[store_eng]
            n_ctile = 2 if half_tiles else 1
            kt = NKCH // n_ctile  # chunks per ct tile
            with loop_ctx:
                for s in range(S_PER_CORE):
                    cts = []
                    for h in range(n_ctile):
                        ct = cpool.tile([128, kt, N_ATOMS], _BF, tag="ct")
                        cts.append(ct)
                        for qq in range(dma_splits // n_ctile):
                            q = h * (dma_splits // n_ctile) + qq
                            ke = ksplit * q + ksplit // dma_frac
                            if packed:
                                src = cb[s, :, ksplit * q : ke]
                            else:
                                r0 = 128 * ksplit * q
                                src = cb[s, r0 : r0 + 128 * (ksplit // dma_frac)].rearrange(
                                    "(c p) a -> p c a", p=128
                                )
                            dma_engs[q % len(dma_engs)].dma_start(
                                out=ct[:, ksplit * qq : ksplit * qq + (ke - ksplit * q)],
                                in_=src,
                            )
                    po = psum.tile([OUT_DEPTH, N_ATOMS], _DT, tag="po")
                    nc.tensor.matmul(
                        po[:],
                        lhsT=fbb_sb[:],
                        rhs=bt_sbs[s][:],
                        start=True,
                        stop=skip_mm,
                    )
                    for k in range(NKCH if not skip_mm else 0):
                        f, nb = k // NB, k % NB
                        nc.tensor.matmul(
                            po[:],
                            lhsT=w_sbs[(s, nb)][:, f, :],
                            rhs=cts[k // kt][:, k % kt, :],
                            start=False,
                            stop=(k == NKCH - 1),
                        )
                    ot = outp.tile([OUT_DEPTH, N_ATOMS], _OT, tag="ot")
                    nc.vector.tensor_copy(out=ot[:], in_=po[:])
                    store_e.dma_start(out=out_t[s], in_=ot[:])

    _hoist_extra_waits(nc)
    return nc


def _hoist_extra_waits(nc):
    """This walrus build rejects any instruction struct carrying more than one
    semaphore wait ("Too many sync wait commands"); Tile freely attaches
    several.  Waits execute in engine-queue order, so hoisting the extras onto
    NoOps inserted directly before the instruction on the same engine is
    semantically identical.  One wait per NoOp."""
    for f in nc.m.functions:
        for blk in f.blocks:
            insts = blk.instructions
            rebuilt = []
            changed = False
            for inst in insts:
                si = inst.sync_info
                if si is not None and len(si.on_wait) > 1:
                    for w in list(si.on_wait)[:-1]:
                        nop = mybir.InstNoOp(
                            name=nc.get_next_instruction_name(),
                            engine=inst.engine,
                            ins=[],
                            outs=[],
                            sync_info=mybir.SyncInfo(on_wait=[w], on_update=[]),
                        )
                        nc.register_instruction(nop)
                        rebuilt.append(nop)
                    inst.sync_info = mybir.SyncInfo(
                        on_wait=[list(si.on_wait)[-1]], on_update=list(si.on_update)
                    )
                    changed = True
                rebuilt.append(inst)
            if changed:
                del insts[:]
                insts.extend(rebuilt)


_CACHED_NC = {}


def _get_nc(repeat=1, version="v1", **kw):
    key = (repeat, version, tuple(sorted(kw.items())))
    if key not in _CACHED_NC:
        if version == "v2":
            _CACHED_NC[key] = _build_bass_v2(**kw)
        elif version == "v3":
            _CACHED_NC[key] = _build_bass_v3(**kw)
        elif version == "v4":
            _CACHED_NC[key] = _build_bass_v4(**kw)
        elif version == "v5":
            _CACHED_NC[key] = _build_bass_v5(**kw)
        elif version == "v6":
            _CACHED_NC[key] = _build_bass_v5(packed=True, **kw)
        else:
            _CACHED_NC[key] = _build_bass(repeat, **kw)
    return _CACHED_NC[key]


def _prep_inputs_v5(node, conn, bond, filters, packed=False):
    """Host-side prep for v5: conn -> [s, (f n), a] bf16 (contraction-major,
    1 KB contiguous runs); bond -> [s, (f j), a] bf16; node/filters fp32."""
    import ml_dtypes

    bf16 = ml_dtypes.bfloat16
    node = np.asarray(node, dtype=np.float32)
    conn = np.asarray(conn, dtype=np.float32)
    bond = np.asarray(bond, dtype=np.float32)
    filters = np.asarray(filters, dtype=np.float32)

    node_t = np.ascontiguousarray(node.transpose(0, 2, 1))
    ftf = np.ascontiguousarray(filters[:, :, :IN_DEPTH].transpose(2, 1, 0)).reshape(
        IN_DEPTH, FL * OUT_DEPTH
    )
    fbb = (
        np.ascontiguousarray(filters[:, :, IN_DEPTH:].transpose(1, 2, 0))
        .reshape(2 * FL, OUT_DEPTH)
        .astype(bf16)
    )
    # cb[s, f*512 + n, a] = conn[s, a, n, f]
    cb = conn.transpose(0, 3, 2, 1).reshape(N_SAMPLES, N_ATOMS * FL, N_ATOMS)
    if packed:
        # [s, (c p), a] -> [s, p, c, a]: one contiguous run per partition
        cb = cb.reshape(N_SAMPLES, NKCH, 128, N_ATOMS).transpose(0, 2, 1, 3)
    btb = bond.transpose(0, 2, 3, 1).reshape(N_SAMPLES, 2 * FL, N_ATOMS)

    in_maps = []
    for k in range(N_CORES):
        s0 = k * S_PER_CORE
        in_maps.append(
            {
                "cb": cb[s0 : s0 + S_PER_CORE].astype(bf16),
                "xt": np.ascontiguousarray(node_t[s0 : s0 + S_PER_CORE]),
                "btb": btb[s0 : s0 + S_PER_CORE].astype(bf16),
                "ftf": ftf,
                "fbb": fbb,
            }
        )
    return in_maps


def _prep_inputs(node, conn, bond, filters):
    """Host-side layout-only prep + per-core sharding."""
    node = np.asarray(node, dtype=np.float32)
    conn = np.asarray(conn, dtype=np.float32)
    bond = np.asarray(bond, dtype=np.float32)
    filters = np.asarray(filters, dtype=np.float32)

    # xt[s, d, n] = node[s, n, d]
    node_t = np.ascontiguousarray(node.transpose(0, 2, 1))
    # bt[s, f*2+j, a] = bond[s, a, f, j]
    bond_t = np.ascontiguousarray(bond.transpose(0, 2, 3, 1)).reshape(
        N_SAMPLES, 2 * FL, N_ATOMS
    )
    # ftf[d, f*64+o] = filters[o, f, d]
    ftf = np.ascontiguousarray(filters[:, :, :IN_DEPTH].transpose(2, 1, 0)).reshape(
        IN_DEPTH, FL * OUT_DEPTH
    )
    # fb[f*2+j, o] = filters[o, f, 64+j]
    fb = np.ascontiguousarray(filters[:, :, IN_DEPTH:].transpose(1, 2, 0)).reshape(
        2 * FL, OUT_DEPTH
    )

    in_maps = []
    for k in range(N_CORES):
        s0 = k * S_PER_CORE
        in_maps.append(
            {
                "c": np.ascontiguousarray(conn[s0 : s0 + S_PER_CORE]),
                "xt": np.ascontiguousarray(node_t[s0 : s0 + S_PER_CORE]),
                "bt": np.ascontiguousarray(bond_t[s0 : s0 + S_PER_CORE]),
                "ftf": ftf,
                "fb": fb,
            }
        )
    return in_maps


# Best-measured variant used by kernel(); see bench logs in the session.
_BEST_VERSION = "v6"


def run(node_property_tensor, connectivity_tensor, bond_property_tensor, filters,
        trace=False, version=None):
    """Run on 8 cores; returns (output [32,512,64], BassKernelResults)."""
    version = version or _BEST_VERSION
    nc = _get_nc(version=version)
    if version == "v5":
        prep = _prep_inputs_v5
    elif version == "v6":
        from functools import partial
        prep = partial(_prep_inputs_v5, packed=True)
    else:
        prep = _prep_inputs
    in_maps = prep(
        node_property_tensor, connectivity_tensor, bond_property_tensor, filters
    )
    res = bass_utils.run_bass_kernel_spmd(
        nc, in_maps, core_ids=list(range(N_CORES)), trace=trace
    )
    out = np.empty((N_SAMPLES, N_ATOMS, OUT_DEPTH), dtype=np.float32)
    for k in range(N_CORES):
        s0 = k * S_PER_CORE
        out[s0 : s0 + S_PER_CORE] = res.results[k]["out_t"].transpose(0, 2, 1).astype(
            np.float32
        )
    return out, res


def kernel(node_property_tensor, connectivity_tensor, bond_property_tensor, filters):
    out, _ = run(
        node_property_tensor, connectivity_tensor, bond_property_tensor, filters
    )
    return out

